# revision 17
# baseline (speedup 1.0000x reference)
import sys
sys.path.insert(0, '/opt/trn_rl_repo')
import numpy as np
import concourse.bass as bass
import concourse.bacc as bacc
import concourse.mybir as mybir
import concourse.tile as tile
from concourse.bass_utils import run_bass_kernel_spmd

N, E0, H = 16384, 262144, 256
P = 128
CH = [0, 2, 6, 11, 16]
GW = 5 * 256
F16NP = np.float16
F16 = mybir.dt.float16
F32 = mybir.dt.float32
BF16 = mybir.dt.bfloat16
F8 = mybir.dt.float8e4
I32 = mybir.dt.int32
S3 = 1.0 / np.sqrt(3.0)
SH = 1.0 / 16.0
S2 = 1.0 / np.sqrt(2.0)
INV06 = 1.0 / 0.6

# packed fp16 weight layout: (name, rows, cols)
WLAYOUT = [
    ("wx1_0", 128, 256), ("wx1_1", 128, 256),
    ("wx2_0", 128, 768), ("wx2_1", 128, 768),
    ("wrbf", 65, 1024),
    ("wvp_0", 128, 512), ("wvp_1", 128, 512),
    ("wxv1_0", 128, 256), ("wxv1_1", 128, 256), ("wxv1_2", 128, 256), ("wxv1_3", 128, 256),
    ("wxv2_0", 128, 768), ("wxv2_1", 128, 768),
    ("o1wv1_0", 128, 256), ("o1wv1_1", 128, 256),
    ("o1wv2_0", 128, 128), ("o1wv2_1", 128, 128),
    ("o1wu1_0", 128, 256), ("o1wu1_1", 128, 256), ("o1wu1_2", 128, 256), ("o1wu1_3", 128, 256),
    ("o1wu2_0", 128, 256), ("o1wu2_1", 128, 256),
    ("o2wv1a", 128, 129),
    ("o2wu1_0", 128, 128), ("o2wu1_1", 128, 128),
    ("o2wu2c", 128, 1),
    ("iotaF", 128, 128),
]
WOFF = {}
_c = 0
for _nm, _r, _w in WLAYOUT:
    WOFF[_nm] = (_c, _r, _w)
    _c += _w
CW = _c
# f32 bias pack: col j per name
BLAYOUT = ["bx1_0", "bx1_1", "bxv1_0", "bxv1_1", "o1bu1_0", "o1bu1_1", "o2bu1"]
CB = len(BLAYOUT)


def _fold(inp):
    blk = {}
    ln_g = inp["ln_g"].astype(np.float64)
    ln_b = inp["ln_b"].astype(np.float64)
    Wx1 = inp["W_x1"].astype(np.float64)
    Wx1f = (ln_g[:, None] * Wx1).astype(np.float32).astype(F16NP)
    blk["wx1_0"], blk["wx1_1"] = Wx1f[:P], Wx1f[P:]
    bx1 = (inp["b_x1"] + (ln_b @ Wx1).astype(np.float32)).astype(np.float32)
    Wx2 = inp["W_x2"].astype(np.float64) * INV06
    Wx2[:, H:2 * H] *= S3 * SH
    Wx2[:, 2 * H:] *= SH
    Wx2f = Wx2.astype(np.float32).astype(F16NP)
    blk["wx2_0"], blk["wx2_1"] = Wx2f[:P], Wx2f[P:]
    Wr = np.vstack([inp["W_rbf"], inp["b_rbf"][None, :]]).astype(np.float32).astype(F16NP)
    blk["wrbf"] = np.concatenate([Wr[:, 0:256], Wr[:, 256:512],
                                  Wr[:, 256:512], Wr[:, 512:768]], axis=1)  # [r1|r2|r2|r3]
    Wvp = inp["W_vp"].astype(np.float64).copy()
    Wvp[:, :H] *= SH
    Wvpf = Wvp.astype(np.float32).astype(F16NP)
    blk["wvp_0"], blk["wvp_1"] = Wvpf[:P], Wvpf[P:]
    Wxv1f = inp["W_xv1"].astype(F16NP)
    for k in range(4):
        blk[f"wxv1_{k}"] = Wxv1f[k * P:(k + 1) * P]
    Wxv2 = inp["W_xv2"].astype(np.float64) * INV06
    Wxv2[:, :2 * H] *= S2
    Wxv2[:, 2 * H:] *= 16.0
    Wxv2f = Wxv2.astype(np.float32).astype(F16NP)
    blk["wxv2_0"], blk["wxv2_1"] = Wxv2f[:P], Wxv2f[P:]
    o1Wv1f = inp["o1_Wv1"].astype(F16NP)
    blk["o1wv1_0"], blk["o1wv1_1"] = o1Wv1f[:P], o1Wv1f[P:]
    o1Wv2f = inp["o1_Wv2"].astype(F16NP)
    blk["o1wv2_0"], blk["o1wv2_1"] = o1Wv2f[:P], o1Wv2f[P:]
    o1Wu1f = inp["o1_Wu1"].astype(F16NP)
    for k in range(4):
        blk[f"o1wu1_{k}"] = o1Wu1f[k * P:(k + 1) * P]
    o1Wu2f = (inp["o1_Wu2"].astype(np.float64) * INV06).astype(np.float32).astype(F16NP)
    blk["o1wu2_0"], blk["o1wu2_1"] = o1Wu2f[:P], o1Wu2f[P:]
    blk["o2wv1a"] = np.hstack([inp["o2_Wv1"], inp["o2_Wv2"]]).astype(F16NP)
    o2Wu1 = inp["o2_Wu1"].astype(np.float64).copy()
    o2Wu1[:P, :] *= INV06
    o2Wu1f = o2Wu1.astype(np.float32).astype(F16NP)
    blk["o2wu1_0"], blk["o2wu1_1"] = o2Wu1f[:P], o2Wu1f[P:]
    blk["o2wu2c"] = (inp["o2_Wu2"][:, 1:2].astype(np.float64) * INV06).astype(np.float32).astype(F16NP)
    blk["iotaF"] = np.broadcast_to(np.arange(P, dtype=F16NP)[None, :], (P, P)).copy()
    wpk = np.zeros((P, CW), F16NP)
    for nm, r, w in WLAYOUT:
        c0 = WOFF[nm][0]
        wpk[:r, c0:c0 + w] = blk[nm]
    bpk = np.zeros((P, CB), np.float32)
    bpk[:, 0] = bx1[:P]
    bpk[:, 1] = bx1[P:]
    bpk[:, 2] = inp["b_xv1"][:P]
    bpk[:, 3] = inp["b_xv1"][P:]
    bpk[:, 4] = inp["o1_bu1"][:P]
    bpk[:, 5] = inp["o1_bu1"][P:]
    bpk[:, 6] = inp["o2_bu1"][:P]
    for nm in ("b_x2", "o1_bu2", "o2_bu2"):
        assert not np.any(inp[nm]), f"nonzero {nm} unsupported by folding"
    return {"wpk": wpk, "bpk": bpk}


def _pack(edge_index, edge_rbf, edge_vector):
    E = edge_index.shape[1]
    src = edge_index[0].astype(np.int64)
    dst = edge_index[1].astype(np.int64)
    gw = dst >> 7
    order = np.argsort(gw, kind="stable")
    gs = gw[order]
    srcs = src[order]
    dsts = dst[order]
    rbfs = edge_rbf[order]
    evs = edge_vector[order]
    cnt = np.bincount(gs, minlength=P)
    T = int(np.ceil(cnt.max() / P))
    NT = 16 * T
    startw = np.concatenate([[0], np.cumsum(cnt)[:-1]])
    r = np.arange(E) - startw[gs]
    core = (gs >> 4).astype(np.int64)
    wl = gs & 15
    tw = r >> 7
    p = r & 127
    t = wl * T + tw
    # remap src node id -> row in the AllGather-ed gfull layout:
    # chunk k holds groups [CH_START[k]:CH_START[k+1]) of every core, core-major
    c_s = srcs >> 11
    j = srcs & 2047
    gi = j >> 7
    rr = j & 127
    CH_START = np.array(CH)
    CH_BASE = CH_START * 8 * 128
    kk = np.searchsorted(CH_START, gi, side="right") - 1
    glen = (CH_START[kk + 1] - CH_START[kk])
    gidx = (CH_BASE[kk] + c_s * glen * 128 + (gi - CH_START[kk]) * 128 + rr).astype(np.int32)
    eidx = np.zeros((8, P, NT), np.int32)
    evp = np.zeros((8, P, 3 * NT), np.float32)
    dstw = np.zeros((8, P, NT), np.float32)
    rbtW = np.zeros((8, 65, NT * P), F16NP)
    eidx[core, p, t] = gidx
    for c in range(3):
        evp[core, p, 3 * t + c] = evs[:, c]
    dstw[core, p, t] = (dsts & 127).astype(np.float32)
    cols = t * P + p
    rbtW[core[:, None], np.arange(64)[None, :], cols[:, None]] = rbfs.astype(F16NP)
    rbtW[core, 64, cols] = 1.0
    return T, NT, eidx, evp, dstw, rbtW


def _build(NT, T, dbg=False):
    A = mybir.AluOpType
    FN = mybir.ActivationFunctionType
    nc = bacc.Bacc("TRN2", target_bir_lowering=False, debug=True, num_devices=8)
    dp = nc.declare_dram_parameter
    xo_d = dp("xown", [P, 16 * H], F32, isOutput=False)
    vo_d = dp("vecown", [P, 16 * 3 * H], F32, isOutput=False)
    ei_d = dp("eidx", [P, NT], I32, isOutput=False)
    ev_d = dp("evp", [P, 3 * NT], F32, isOutput=False)
    dw_d = dp("dstw", [P, NT], F32, isOutput=False)
    rb_d = dp("rbtW", [65, NT * P], F16, isOutput=False)
    wpk_d = dp("wpk", [P, CW], F16, isOutput=False)
    bpk_d = dp("bpk", [P, CB], F32, isOutput=False)
    out_d = dp("outT", [4, 2048], F32, isOutput=True)

    with tile.TileContext(nc) as tc:
        with tc.tile_pool(name="persist", bufs=1) as PR, \
             tc.tile_pool(name="dpool", bufs=1, space="DRAM") as DP:
            wpk = PR.tile([P, CW], F16, tag="wpk", name="wpk")
            nc.sync.dma_start(out=wpk[:], in_=wpk_d[:, :])
            bpk = PR.tile([P, CB], F32, tag="bpk", name="bpk")
            nc.sync.dma_start(out=bpk[:], in_=bpk_d[:, :])
            xot = PR.tile([P, 16 * H], F32, tag="xot", name="xot")
            nc.sync.dma_start(out=xot[:], in_=xo_d[:, :])
            vot = PR.tile([P, 16 * 3 * H], F32, tag="vot", name="vot")
            nc.scalar.dma_start(out=vot[:], in_=vo_d[:, :])

            def W(nm):
                c0, r, w = WOFF[nm]
                return wpk[0:r, c0:c0 + w]

            def B(nm):
                j = BLAYOUT.index(nm)
                return bpk[:, j:j + 1]

            wx1 = [W("wx1_0"), W("wx1_1")]
            bx1 = [B("bx1_0"), B("bx1_1")]
            wx2 = [W("wx2_0"), W("wx2_1")]
            wrbf = W("wrbf")
            wvp = [W("wvp_0"), W("wvp_1")]
            wxv1 = [W(f"wxv1_{k}") for k in range(4)]
            bxv1 = [B("bxv1_0"), B("bxv1_1")]
            wxv2 = [W("wxv2_0"), W("wxv2_1")]
            o1wv1 = [W("o1wv1_0"), W("o1wv1_1")]
            o1wv2 = [W("o1wv2_0"), W("o1wv2_1")]
            o1wu1 = [W(f"o1wu1_{k}") for k in range(4)]
            o1bu1 = [B("o1bu1_0"), B("o1bu1_1")]
            o1wu2 = [W("o1wu2_0"), W("o1wu2_1")]
            o2wv1a = W("o2wv1a")
            o2wu1 = [W("o2wu1_0"), W("o2wu1_1")]
            o2bu1 = B("o2bu1")
            o2wu2c = W("o2wu2c")
            iotaF = W("iotaF")
            eidx = PR.tile([P, NT], I32, tag="eidx", name="eidx")
            nc.scalar.dma_start(out=eidx[:], in_=ei_d[:, :])
            evp = PR.tile([P, 3 * NT], F32, tag="evp", name="evp")
            nc.scalar.dma_start(out=evp[:], in_=ev_d[:, :])
            dstw = PR.tile([P, NT], F32, tag="dstw", name="dstw")
            nc.scalar.dma_start(out=dstw[:], in_=dw_d[:, :])
            eps5 = PR.tile([P, 1], F32, tag="eps5", name="eps5")
            nc.vector.memset(eps5[:], 1e-5)
            eps8 = PR.tile([P, 1], F32, tag="eps8", name="eps8")
            nc.vector.memset(eps8[:], 1e-8)

            CH_START = CH
            gown = [DP.tile([(CH_START[k + 1] - CH_START[k]) * P, GW], F16,
                            tag=f"gown{k}", name=f"gown{k}") for k in range(4)]
            gfull = nc.dram_tensor("gfull", [N, GW], F16, addr_space="Shared")

            def xo(i):
                return xot[:, i * H:(i + 1) * H]

            def vo(i):
                return vot[:, i * 3 * H:(i + 1) * 3 * H]

            def rep3(ap):
                return ap.rearrange("p (o f) -> p o f", o=1).broadcast_to([P, 3, H])

            # ---------------- phase 1: g-pack own nodes + AllGather ----------------
            with tc.tile_pool(name="p1", bufs=2) as S1, \
                 tc.tile_pool(name="q1", bufs=2, space="PSUM") as Q1:
                mvs = []
                for i in range(16):
                    st6 = S1.tile([P, 6], F32, tag="st6", bufs=3)
                    nc.vector.bn_stats(out=st6[:], in_=xo(i))
                    mv = S1.tile([P, 2], F32, tag=f"mv{i}", name=f"mv{i}")
                    nc.vector.bn_aggr(out=mv[:], in_=st6[:])
                    mvs.append(mv)
                sds = []
                for i in range(16):
                    sd = S1.tile([P, 1], F32, tag=f"sd{i}")
                    nc.scalar.activation(out=sd[:], in_=mvs[i][:, 1:2], func=FN.Sqrt,
                                         bias=eps5[:, 0:1])
                    sds.append(sd)
                for i in range(16):
                    ri = S1.tile([P, 1], F32, tag="ri", bufs=3)
                    nc.vector.reciprocal(out=ri[:], in_=sds[i][:])
                    xh = S1.tile([P, H], F16, tag="xh", bufs=4)
                    nc.vector.tensor_scalar(out=xh[:], in0=xo(i), scalar1=mvs[i][:, 0:1],
                                            scalar2=ri[:, 0:1], op0=A.subtract, op1=A.mult)
                    xhT = S1.tile([P, H], F16, tag="xhT", bufs=4)
                    for k in range(2):
                        (nc.sync if k == 0 else nc.scalar).dma_start_transpose(
                            out=xhT[:, k * P:(k + 1) * P], in_=xh[:, k * P:(k + 1) * P])
                    hT = Q1.tile([P, H], F32, tag="p1h", bufs=3)
                    for mo in range(2):
                        for k in range(2):
                            nc.tensor.matmul(out=hT[:, mo * P:(mo + 1) * P],
                                             lhsT=wx1[k][:, mo * P:(mo + 1) * P],
                                             rhs=xhT[:, k * P:(k + 1) * P],
                                             start=(k == 0), stop=(k == 1))
                    silT = S1.tile([P, H], F16, tag="silT", bufs=4)
                    for mo in range(2):
                        nc.scalar.activation(out=silT[:, mo * P:(mo + 1) * P],
                                             in_=hT[:, mo * P:(mo + 1) * P],
                                             func=FN.Silu, bias=bx1[mo][:, 0:1])
                    pA = Q1.tile([P, 2 * H], F32, tag="p1a", bufs=3)
                    pB = Q1.tile([P, H], F32, tag="p1b", bufs=2)
                    for mo in range(2):
                        nc.tensor.matmul(out=pA[:], lhsT=silT[:, mo * P:(mo + 1) * P],
                                         rhs=wx2[mo][:, 0:2 * H], start=(mo == 0), stop=(mo == 1))
                        nc.tensor.matmul(out=pB[:], lhsT=silT[:, mo * P:(mo + 1) * P],
                                         rhs=wx2[mo][:, 2 * H:3 * H], start=(mo == 0), stop=(mo == 1))
                    gp = S1.tile([P, GW], F16, tag="gp", bufs=4)
                    nc.scalar.activation(out=gp[:, 0:H], in_=pA[:, 0:H], func=FN.Copy)
                    nc.vector.tensor_tensor(
                        out=gp[:, H:4 * H].rearrange("p (c f) -> p c f", c=3),
                        in0=vo(i).rearrange("p (c f) -> p c f", c=3),
                        in1=rep3(pA[:, H:2 * H]), op=A.mult)
                    nc.scalar.activation(out=gp[:, 4 * H:5 * H], in_=pB[:, 0:H], func=FN.Copy)
                    k = next(kk for kk in range(4) if CH_START[kk] <= i < CH_START[kk + 1])
                    gi0 = i - CH_START[k]
                    nc.gpsimd.dma_start(out=gown[k][gi0 * P:(gi0 + 1) * P, :], in_=gp[:])
                    if i == CH_START[k + 1] - 1:
                        nc.gpsimd.collective_compute(
                            "AllGather", A.bypass, replica_groups=[list(range(8))],
                            ins=[gown[k][:].opt()],
                            outs=[gfull[CH_START[k] * 1024:CH_START[k + 1] * 1024, :].opt()])

            # -------- fused phase 2 (messages + scatter) / phase 3 (update + output) --------
            with tc.tile_pool(name="p2", bufs=2) as S2, \
                 tc.tile_pool(name="q2", bufs=2, space="PSUM") as Q2:
                for pr in range(8):
                    # pair-level transposed tiles: [128 feat, 256 nodes]
                    x1T = S2.tile([P, 2 * H], F16, tag="x1T", bufs=2)
                    v1T = [S2.tile([P, 2 * H], F16, tag=f"v1T{c}", bufs=2, name=f"v1T{c}_{pr}") for c in range(3)]
                    for wi in range(2):
                        w = 2 * pr + wi
                        winAB = Q2.tile([P, 4 * H], F32, tag="winAB", bufs=1)
                        rbtWt = S2.tile([65, T * P], F16, tag="rbtW", bufs=2)
                        nc.sync.dma_start(out=rbtWt[:], in_=rb_d[:, w * T * P:(w + 1) * T * P])
                        for tw in range(T):
                            t = w * T + tw
                            gt = S2.tile([P, GW], F16, tag="gt", bufs=3)
                            nc.gpsimd.indirect_dma_start(
                                out=gt[:], out_offset=None, in_=gfull[:],
                                in_offset=bass.IndirectOffsetOnAxis(ap=eidx[:, t:t + 1], axis=0))
                            stt = S2.tile([P, P], F16, tag="stt", bufs=4)
                            nc.vector.tensor_scalar(out=stt[:], in0=iotaF,
                                                    scalar1=dstw[:, t:t + 1], scalar2=None,
                                                    op0=A.is_equal)
                            rAx = Q2.tile([P, 4 * H], F32, tag="rAx", bufs=2)
                            nc.tensor.matmul(out=rAx[:, 0:512], lhsT=rbtWt[:, tw * P:(tw + 1) * P],
                                             rhs=wrbf[:, 0:512], start=True, stop=True)
                            nc.tensor.matmul(out=rAx[:, 512:1024], lhsT=rbtWt[:, tw * P:(tw + 1) * P],
                                             rhs=wrbf[:, 512:1024], start=True, stop=True)
                            rsb = S2.tile([P, 4 * H], F16, tag="rsb", bufs=3)
                            nc.scalar.activation(out=rsb[:], in_=rAx[:], func=FN.Copy)
                            mall = S2.tile([P, 7 * H], F16, tag="mall", bufs=3)
                            nc.vector.tensor_tensor(out=mall[:, 0:2 * H], in0=gt[:, H:3 * H],
                                                    in1=rsb[:, H:3 * H], op=A.mult)
                            nc.vector.tensor_tensor(out=mall[:, 2 * H:3 * H],
                                                    in0=gt[:, 3 * H:4 * H],
                                                    in1=rsb[:, H:2 * H], op=A.mult)
                            nc.vector.tensor_tensor(out=mall[:, 3 * H:4 * H],
                                                    in0=gt[:, 0:H],
                                                    in1=rsb[:, 0:H], op=A.mult)
                            nc.vector.tensor_tensor(out=mall[:, 4 * H:5 * H],
                                                    in0=gt[:, 4 * H:5 * H],
                                                    in1=rsb[:, 3 * H:4 * H], op=A.mult)
                            for c in (1, 2):
                                nc.vector.tensor_scalar(
                                    out=mall[:, (4 + c) * H:(5 + c) * H],
                                    in0=mall[:, 4 * H:5 * H],
                                    scalar1=evp[:, 3 * t + c:3 * t + c + 1], scalar2=None,
                                    op0=A.mult)
                            nc.vector.tensor_scalar(
                                out=mall[:, 4 * H:5 * H], in0=mall[:, 4 * H:5 * H],
                                scalar1=evp[:, 3 * t:3 * t + 1], scalar2=None,
                                op0=A.mult)
                            st0, sp0 = (tw == 0), (tw == T - 1)
                            # m3c scatter first: start=True clears each bank once (tw 0)
                            nc.tensor.matmul(out=winAB[:, 0:512], lhsT=stt[:],
                                             rhs=mall[:, 1024:1536], start=st0, stop=False)
                            nc.tensor.matmul(out=winAB[:, 512:768], lhsT=stt[:],
                                             rhs=mall[:, 1536:1792], start=st0, stop=False)
                            # mv/m1 scatter accumulates into the same columns
                            nc.tensor.matmul(out=winAB[:, 0:512], lhsT=stt[:],
                                             rhs=mall[:, 0:512], start=False, stop=sp0)
                            nc.tensor.matmul(out=winAB[:, 512:1024], lhsT=stt[:],
                                             rhs=mall[:, 512:1024], start=False, stop=sp0)
                        x1 = S2.tile([P, H], F16, tag="x1", bufs=2)
                        nc.vector.tensor_tensor(out=x1[:], in0=xo(w), in1=winAB[:, 3 * H:4 * H],
                                                op=A.add)
                        v1 = S2.tile([P, 3 * H], F16, tag="v1", bufs=2)
                        nc.vector.tensor_tensor(out=v1[:], in0=vo(w), in1=winAB[:, 0:3 * H],
                                                op=A.add)
                        for k in range(2):
                            nc.sync.dma_start_transpose(
                                out=x1T[:, k * 2 * P + wi * P:k * 2 * P + (wi + 1) * P],
                                in_=x1[:, k * P:(k + 1) * P])
                        for c in range(3):
                            eng = nc.sync
                            for k in range(2):
                                eng.dma_start_transpose(
                                    out=v1T[c][:, k * 2 * P + wi * P:k * 2 * P + (wi + 1) * P],
                                    in_=v1[:, c * H + k * P:c * H + (k + 1) * P])

                    # ---------------- phase 3 on the pair (256 node columns) ----------------
                    n2 = 2 * P
                    ppn = [0]
                    def pp(cols):
                        ppn[0] += 1
                        return Q2.tile([P, cols], F32, tag="pp", bufs=2,
                                       name=f"pp_{pr}_{ppn[0]}")
                    # vp = vec @ Wvp (transposed out), vd, vnorm
                    vp1sb = []
                    sqs = []
                    prods = []
                    for c in range(3):
                        vp1_ps = pp(2 * H)
                        vp2_ps = pp(2 * H)
                        for m in range(4):
                            dst_ps = vp1_ps if m < 2 else vp2_ps
                            mm = m % 2
                            for k in range(2):
                                nc.tensor.matmul(
                                    out=dst_ps[:, mm * n2:(mm + 1) * n2],
                                    lhsT=wvp[k][:, m * P:(m + 1) * P],
                                    rhs=v1T[c][:, k * n2:(k + 1) * n2],
                                    start=(k == 0), stop=(k == 1))
                        v1sb = S2.tile([P, 2 * H], F16, tag=f"vp1sb{c}", bufs=2)
                        nc.scalar.activation(out=v1sb[:], in_=vp1_ps[:], func=FN.Copy)
                        vp1sb.append(v1sb)
                        prod = S2.tile([P, 2 * H], F16, tag=f"prod{c}", bufs=1)
                        nc.vector.tensor_tensor(out=prod[:], in0=v1sb[:], in1=vp2_ps[:],
                                                op=A.mult)
                        prods.append(prod)
                        sq = S2.tile([P, 2 * H], F32, tag=f"sq{c}", bufs=1)
                        nc.scalar.activation(out=sq[:], in_=vp2_ps[:], func=FN.Square)
                        sqs.append(sq)
                    vns = S2.tile([P, 2 * H], F32, tag="vns", bufs=1)
                    nc.vector.tensor_tensor(out=vns[:], in0=sqs[0][:], in1=sqs[1][:], op=A.add)
                    nc.vector.tensor_tensor(out=vns[:], in0=vns[:], in1=sqs[2][:], op=A.add)
                    vnT = S2.tile([P, 2 * H], F16, tag="vnT", bufs=2)
                    nc.scalar.activation(out=vnT[:], in_=vns[:], func=FN.Sqrt, bias=eps8[:, 0:1])
                    # MLP-U: h = silu([x1|vn] @ Wxv1 + b)
                    hT_ps = pp(2 * H)
                    ins4 = [x1T, x1T, vnT, vnT]
                    for mo in range(2):
                        for kk in range(4):
                            nc.tensor.matmul(
                                out=hT_ps[:, mo * n2:(mo + 1) * n2],
                                lhsT=wxv1[kk][:, mo * P:(mo + 1) * P],
                                rhs=ins4[kk][:, (kk % 2) * n2:(kk % 2 + 1) * n2],
                                start=(kk == 0), stop=(kk == 3))
                    silT2 = S2.tile([P, 2 * H], F16, tag="silT2", bufs=2)
                    for mo in range(2):
                        nc.scalar.activation(out=silT2[:, mo * n2:(mo + 1) * n2],
                                             in_=hT_ps[:, mo * n2:(mo + 1) * n2],
                                             func=FN.Silu, bias=bxv1[mo][:, 0:1])
                    # xvh = silu @ Wxv2  (xv1: chunks 0-1, xv2: 2-3, xv3: 4-5)
                    xv1_ps = pp(2 * H)
                    xv2_ps = pp(2 * H)
                    xv3_ps = pp(2 * H)
                    for j in range(6):
                        dst_ps = (xv1_ps, xv2_ps, xv3_ps)[j // 2]
                        out_ap = dst_ps[:, (j % 2) * n2:(j % 2 + 1) * n2]
                        for mo in range(2):
                            nc.tensor.matmul(out=out_ap,
                                             lhsT=wxv2[mo][:, j * P:(j + 1) * P],
                                             rhs=silT2[:, mo * n2:(mo + 1) * n2],
                                             start=(mo == 0), stop=(mo == 1))
                    # vdT = sum_c vp1T_c * vp2T_c   (per node/feature, channels summed)
                    vdt = S2.tile([P, 2 * H], F16, tag="vdt", bufs=1)
                    nc.vector.tensor_tensor(out=vdt[:], in0=prods[0][:], in1=prods[1][:],
                                            op=A.add)
                    vdT = S2.tile([P, 2 * H], F16, tag="vdT", bufs=2)
                    nc.vector.tensor_tensor(out=vdT[:], in0=vdt[:], in1=prods[2][:], op=A.add)
                    # x2 = x1 + xv1 + xv2*vd
                    xv2sb = S2.tile([P, 2 * H], F16, tag="xv2sb", bufs=2)
                    nc.scalar.activation(out=xv2sb[:], in_=xv2_ps[:], func=FN.Copy)
                    tg = S2.tile([P, 2 * H], F16, tag="tg", bufs=2)
                    nc.vector.tensor_tensor(out=tg[:], in0=xv2sb[:], in1=vdT[:], op=A.mult)
                    ux = S2.tile([P, 2 * H], F16, tag="ux", bufs=2)
                    nc.vector.tensor_tensor(out=ux[:], in0=x1T[:], in1=xv1_ps[:], op=A.add)
                    x2T = S2.tile([P, 2 * H], F16, tag="x2T", bufs=2)
                    nc.vector.tensor_tensor(out=x2T[:], in0=ux[:], in1=tg[:], op=A.add)
                    # vec2_c = v1_c + xv3 * vp1_c
                    vec2T = []
                    for c in range(3):
                        s = S2.tile([P, 2 * H], F16, tag="s3", bufs=3)
                        nc.vector.tensor_tensor(out=s[:], in0=vp1sb[c][:], in1=xv3_ps[:],
                                                op=A.mult)
                        v2 = S2.tile([P, 2 * H], F16, tag=f"vec2T{c}", bufs=2)
                        nc.vector.tensor_tensor(out=v2[:], in0=v1T[c][:], in1=s[:], op=A.add)
                        vec2T.append(v2)
                    # ---- gated block 1 ----
                    sq1s = []
                    for c in range(3):
                        pw1_ps = pp(2 * H)
                        for m in range(2):
                            for k in range(2):
                                nc.tensor.matmul(out=pw1_ps[:, m * n2:(m + 1) * n2],
                                                 lhsT=o1wv1[k][:, m * P:(m + 1) * P],
                                                 rhs=vec2T[c][:, k * n2:(k + 1) * n2],
                                                 start=(k == 0), stop=(k == 1))
                        sq1 = S2.tile([P, 2 * H], F32, tag=f"sq1_{c}", bufs=1)
                        nc.scalar.activation(out=sq1[:], in_=pw1_ps[:], func=FN.Square)
                        sq1s.append(sq1)
                    vns1 = S2.tile([P, 2 * H], F32, tag="vns1", bufs=1)
                    nc.vector.tensor_tensor(out=vns1[:], in0=sq1s[0][:], in1=sq1s[1][:], op=A.add)
                    nc.vector.tensor_tensor(out=vns1[:], in0=vns1[:], in1=sq1s[2][:], op=A.add)
                    vn1T = S2.tile([P, 2 * H], F16, tag="vn1T", bufs=2)
                    nc.scalar.activation(out=vn1T[:], in_=vns1[:], func=FN.Sqrt)
                    hT1_ps = pp(2 * H)
                    ins1 = [x2T, x2T, vn1T, vn1T]
                    for mo in range(2):
                        for kk in range(4):
                            nc.tensor.matmul(
                                out=hT1_ps[:, mo * n2:(mo + 1) * n2],
                                lhsT=o1wu1[kk][:, mo * P:(mo + 1) * P],
                                rhs=ins1[kk][:, (kk % 2) * n2:(kk % 2 + 1) * n2],
                                start=(kk == 0), stop=(kk == 3))
                    sil1T = S2.tile([P, 2 * H], F16, tag="sil1T", bufs=2)
                    for mo in range(2):
                        nc.scalar.activation(out=sil1T[:, mo * n2:(mo + 1) * n2],
                                             in_=hT1_ps[:, mo * n2:(mo + 1) * n2],
                                             func=FN.Silu, bias=o1bu1[mo][:, 0:1])
                    ph1_ps = pp(2 * H)
                    for j in range(2):
                        for mo in range(2):
                            nc.tensor.matmul(out=ph1_ps[:, j * n2:(j + 1) * n2],
                                             lhsT=o1wu2[mo][:, j * P:(j + 1) * P],
                                             rhs=sil1T[:, mo * n2:(mo + 1) * n2],
                                             start=(mo == 0), stop=(mo == 1))
                    xnT = S2.tile([P, n2], F16, tag="xnT", bufs=2)
                    nc.scalar.activation(out=xnT[:], in_=ph1_ps[:, 0:n2], func=FN.Silu)
                    gate = S2.tile([P, n2], F16, tag="gate", bufs=2)
                    nc.scalar.activation(out=gate[:], in_=ph1_ps[:, n2:2 * n2], func=FN.Copy)
                    vnbT = []
                    for c in range(3):
                        v2b_ps = pp(n2)
                        for k in range(2):
                            nc.tensor.matmul(out=v2b_ps[:],
                                             lhsT=o1wv2[k][:],
                                             rhs=vec2T[c][:, k * n2:(k + 1) * n2],
                                             start=(k == 0), stop=(k == 1))
                        vb = S2.tile([P, n2], F16, tag=f"vnbT{c}", bufs=2)
                        nc.vector.tensor_tensor(out=vb[:], in0=v2b_ps[:], in1=gate[:], op=A.mult)
                        vnbT.append(vb)
                    # ---- gated block 2 ----
                    v2fsb = S2.tile([1, 3 * n2], F16, tag="v2fsb", bufs=2)
                    sq2s = []
                    for c in range(3):
                        pw2_ps = pp(n2)
                        nc.tensor.matmul(out=pw2_ps[:], lhsT=o2wv1a[:, 0:P], rhs=vnbT[c][:],
                                         start=True, stop=True)
                        v2f_ps = pp(n2)[0:1, :]
                        nc.tensor.matmul(out=v2f_ps, lhsT=o2wv1a[:, P:P + 1], rhs=vnbT[c][:],
                                         start=True, stop=True)
                        nc.scalar.activation(out=v2fsb[0:1, c * n2:(c + 1) * n2], in_=v2f_ps,
                                             func=FN.Copy)
                        sq2 = S2.tile([P, n2], F32, tag=f"sq2_{c}", bufs=1)
                        nc.scalar.activation(out=sq2[:], in_=pw2_ps[:], func=FN.Square)
                        sq2s.append(sq2)
                    vns2 = S2.tile([P, n2], F32, tag="vns2", bufs=1)
                    nc.vector.tensor_tensor(out=vns2[:], in0=sq2s[0][:], in1=sq2s[1][:], op=A.add)
                    nc.vector.tensor_tensor(out=vns2[:], in0=vns2[:], in1=sq2s[2][:], op=A.add)
                    vn2T = S2.tile([P, n2], F16, tag="vn2T", bufs=2)
                    nc.scalar.activation(out=vn2T[:], in_=vns2[:], func=FN.Sqrt)
                    h2_ps = pp(n2)
                    nc.tensor.matmul(out=h2_ps[:], lhsT=o2wu1[0][:], rhs=xnT[:],
                                     start=True, stop=False)
                    nc.tensor.matmul(out=h2_ps[:], lhsT=o2wu1[1][:], rhs=vn2T[:],
                                     start=False, stop=True)
                    sil2T = S2.tile([P, n2], F16, tag="sil2T", bufs=2)
                    nc.scalar.activation(out=sil2T[:], in_=h2_ps[:], func=FN.Silu,
                                         bias=o2bu1[:, 0:1])
                    phb_ps = pp(n2)[0:1, :]
                    nc.tensor.matmul(out=phb_ps, lhsT=o2wu2c[:], rhs=sil2T[:],
                                     start=True, stop=True)
                    hbs = S2.tile([1, n2], F16, tag="hbs", bufs=2)
                    nc.scalar.activation(out=hbs[:], in_=phb_ps, func=FN.Copy)
                    otp = S2.tile([P, n2], F32, tag="otp", bufs=2)
                    for c in range(3):
                        nc.vector.tensor_tensor(out=otp[32 * c:32 * c + 1, :],
                                                in0=v2fsb[0:1, c * n2:(c + 1) * n2],
                                                in1=hbs[0:1, :], op=A.mult)
                        nc.sync.dma_start(out=out_d[c:c + 1, pr * n2:(pr + 1) * n2],
                                          in_=otp[32 * c:32 * c + 1, :])

    nc.compile()
    return nc


def _make_inputs(inputs):
    f = _fold(inputs)
    T, NT, eidx, evp, dstw, rbtW = _pack(
        inputs["edge_index"], inputs["edge_rbf"], inputs["edge_vector"])
    x = np.asarray(inputs["x"], np.float32)
    vecf = np.asarray(inputs["vec"], np.float32).reshape(N, 3 * H)
    ins = []
    for c in range(8):
        xs = x[c * 2048:(c + 1) * 2048].reshape(16, P, H).transpose(1, 0, 2).reshape(P, 16 * H)
        vs = vecf[c * 2048:(c + 1) * 2048].reshape(16, P, 3 * H).transpose(1, 0, 2).reshape(P, 16 * 3 * H)
        d = {
            "xown": np.ascontiguousarray(xs), "vecown": np.ascontiguousarray(vs),
            "eidx": eidx[c], "evp": evp[c], "dstw": dstw[c], "rbtW": rbtW[c],
        }
        d.update(f)
        ins.append(d)
    return T, NT, ins


def kernel(**inputs):
    T, NT, ins = _make_inputs(inputs)
    nc = _build(NT, T)
    res = run_bass_kernel_spmd(nc, ins, list(range(8)))
    out = np.concatenate([res.results[c]["outT"][:3].T for c in range(8)], axis=0)
    return np.ascontiguousarray(out.astype(np.float32))


# revision 18
# speedup vs baseline: 1.1223x; 1.1223x over previous
import sys
sys.path.insert(0, '/opt/trn_rl_repo')
import numpy as np
import concourse.bass as bass
import concourse.bacc as bacc
import concourse.mybir as mybir
import concourse.tile as tile
from concourse.bass_utils import run_bass_kernel_spmd

N, E0, H = 16384, 262144, 256
P = 128
CH = [0, 2, 6, 11, 16]
GW = 5 * 256
F16NP = np.float16
F16 = mybir.dt.float16
F32 = mybir.dt.float32
BF16 = mybir.dt.bfloat16
F8 = mybir.dt.float8e4
I32 = mybir.dt.int32
S3 = 1.0 / np.sqrt(3.0)
SH = 1.0 / 16.0
S2 = 1.0 / np.sqrt(2.0)
INV06 = 1.0 / 0.6

# packed fp16 weight layout: (name, rows, cols)
WLAYOUT = [
    ("wx1_0", 128, 256), ("wx1_1", 128, 256),
    ("wx2_0", 128, 768), ("wx2_1", 128, 768),
    ("wrbf", 65, 1024),
    ("wvp_0", 128, 512), ("wvp_1", 128, 512),
    ("wxv1_0", 128, 256), ("wxv1_1", 128, 256), ("wxv1_2", 128, 256), ("wxv1_3", 128, 256),
    ("wxv2_0", 128, 768), ("wxv2_1", 128, 768),
    ("o1wv1_0", 128, 256), ("o1wv1_1", 128, 256),
    ("o1wv2_0", 128, 128), ("o1wv2_1", 128, 128),
    ("o1wu1_0", 128, 256), ("o1wu1_1", 128, 256), ("o1wu1_2", 128, 256), ("o1wu1_3", 128, 256),
    ("o1wu2_0", 128, 256), ("o1wu2_1", 128, 256),
    ("o2wv1a", 128, 129),
    ("o2wu1_0", 128, 128), ("o2wu1_1", 128, 128),
    ("o2wu2c", 128, 1),
    ("iotaF", 128, 128),
]
WOFF = {}
_c = 0
for _nm, _r, _w in WLAYOUT:
    WOFF[_nm] = (_c, _r, _w)
    _c += _w
CW = _c
# f32 bias pack: col j per name
BLAYOUT = ["bx1_0", "bx1_1", "bxv1_0", "bxv1_1", "o1bu1_0", "o1bu1_1", "o2bu1"]
CB = len(BLAYOUT)


def _fold(inp):
    blk = {}
    ln_g = inp["ln_g"].astype(np.float64)
    ln_b = inp["ln_b"].astype(np.float64)
    Wx1 = inp["W_x1"].astype(np.float64)
    Wx1f = (ln_g[:, None] * Wx1).astype(np.float32).astype(F16NP)
    blk["wx1_0"], blk["wx1_1"] = Wx1f[:P], Wx1f[P:]
    bx1 = (inp["b_x1"] + (ln_b @ Wx1).astype(np.float32)).astype(np.float32)
    Wx2 = inp["W_x2"].astype(np.float64) * INV06
    Wx2[:, H:2 * H] *= S3 * SH
    Wx2[:, 2 * H:] *= SH
    Wx2f = Wx2.astype(np.float32).astype(F16NP)
    blk["wx2_0"], blk["wx2_1"] = Wx2f[:P], Wx2f[P:]
    Wr = np.vstack([inp["W_rbf"], inp["b_rbf"][None, :]]).astype(np.float32).astype(F16NP)
    blk["wrbf"] = np.concatenate([Wr[:, 0:256], Wr[:, 256:512],
                                  Wr[:, 256:512], Wr[:, 512:768]], axis=1)  # [r1|r2|r2|r3]
    Wvp = inp["W_vp"].astype(np.float64).copy()
    Wvp[:, :H] *= SH
    Wvpf = Wvp.astype(np.float32).astype(F16NP)
    blk["wvp_0"], blk["wvp_1"] = Wvpf[:P], Wvpf[P:]
    Wxv1f = inp["W_xv1"].astype(F16NP)
    for k in range(4):
        blk[f"wxv1_{k}"] = Wxv1f[k * P:(k + 1) * P]
    Wxv2 = inp["W_xv2"].astype(np.float64) * INV06
    Wxv2[:, :2 * H] *= S2
    Wxv2[:, 2 * H:] *= 16.0
    Wxv2f = Wxv2.astype(np.float32).astype(F16NP)
    blk["wxv2_0"], blk["wxv2_1"] = Wxv2f[:P], Wxv2f[P:]
    o1Wv1f = inp["o1_Wv1"].astype(F16NP)
    blk["o1wv1_0"], blk["o1wv1_1"] = o1Wv1f[:P], o1Wv1f[P:]
    o1Wv2f = inp["o1_Wv2"].astype(F16NP)
    blk["o1wv2_0"], blk["o1wv2_1"] = o1Wv2f[:P], o1Wv2f[P:]
    o1Wu1f = inp["o1_Wu1"].astype(F16NP)
    for k in range(4):
        blk[f"o1wu1_{k}"] = o1Wu1f[k * P:(k + 1) * P]
    o1Wu2f = (inp["o1_Wu2"].astype(np.float64) * INV06).astype(np.float32).astype(F16NP)
    blk["o1wu2_0"], blk["o1wu2_1"] = o1Wu2f[:P], o1Wu2f[P:]
    blk["o2wv1a"] = np.hstack([inp["o2_Wv1"], inp["o2_Wv2"]]).astype(F16NP)
    o2Wu1 = inp["o2_Wu1"].astype(np.float64).copy()
    o2Wu1[:P, :] *= INV06
    o2Wu1f = o2Wu1.astype(np.float32).astype(F16NP)
    blk["o2wu1_0"], blk["o2wu1_1"] = o2Wu1f[:P], o2Wu1f[P:]
    blk["o2wu2c"] = (inp["o2_Wu2"][:, 1:2].astype(np.float64) * INV06).astype(np.float32).astype(F16NP)
    blk["iotaF"] = np.broadcast_to(np.arange(P, dtype=F16NP)[None, :], (P, P)).copy()
    wpk = np.zeros((P, CW), F16NP)
    for nm, r, w in WLAYOUT:
        c0 = WOFF[nm][0]
        wpk[:r, c0:c0 + w] = blk[nm]
    bpk = np.zeros((P, CB), np.float32)
    bpk[:, 0] = bx1[:P]
    bpk[:, 1] = bx1[P:]
    bpk[:, 2] = inp["b_xv1"][:P]
    bpk[:, 3] = inp["b_xv1"][P:]
    bpk[:, 4] = inp["o1_bu1"][:P]
    bpk[:, 5] = inp["o1_bu1"][P:]
    bpk[:, 6] = inp["o2_bu1"][:P]
    for nm in ("b_x2", "o1_bu2", "o2_bu2"):
        assert not np.any(inp[nm]), f"nonzero {nm} unsupported by folding"
    return {"wpk": wpk, "bpk": bpk}


def _pack(edge_index, edge_rbf, edge_vector):
    E = edge_index.shape[1]
    src = edge_index[0].astype(np.int64)
    dst = edge_index[1].astype(np.int64)
    gw = dst >> 7
    order = np.argsort(gw, kind="stable")
    gs = gw[order]
    srcs = src[order]
    dsts = dst[order]
    rbfs = edge_rbf[order]
    evs = edge_vector[order]
    cnt = np.bincount(gs, minlength=P)
    T = int(np.ceil(cnt.max() / P))
    NT = 16 * T
    startw = np.concatenate([[0], np.cumsum(cnt)[:-1]])
    r = np.arange(E) - startw[gs]
    core = (gs >> 4).astype(np.int64)
    wl = gs & 15
    tw = r >> 7
    p = r & 127
    t = wl * T + tw
    # remap src node id -> row in the AllGather-ed gfull layout:
    # chunk k holds groups [CH_START[k]:CH_START[k+1]) of every core, core-major
    c_s = srcs >> 11
    j = srcs & 2047
    gi = j >> 7
    rr = j & 127
    CH_START = np.array(CH)
    CH_BASE = CH_START * 8 * 128
    kk = np.searchsorted(CH_START, gi, side="right") - 1
    glen = (CH_START[kk + 1] - CH_START[kk])
    gidx = (CH_BASE[kk] + c_s * glen * 128 + (gi - CH_START[kk]) * 128 + rr).astype(np.int32)
    eidx = np.zeros((8, P, NT), np.int32)
    evp = np.zeros((8, P, 3 * NT), np.float32)
    dstw = np.zeros((8, P, NT), np.float32)
    rbtW = np.zeros((8, 65, NT * P), F16NP)
    eidx[core, p, t] = gidx
    for c in range(3):
        evp[core, p, 3 * t + c] = evs[:, c]
    dstw[core, p, t] = (dsts & 127).astype(np.float32)
    cols = t * P + p
    rbtW[core[:, None], np.arange(64)[None, :], cols[:, None]] = rbfs.astype(F16NP)
    rbtW[core, 64, cols] = 1.0
    return T, NT, eidx, evp, dstw, rbtW


def _build(NT, T, dbg=False):
    A = mybir.AluOpType
    FN = mybir.ActivationFunctionType
    nc = bacc.Bacc("TRN2", target_bir_lowering=False, debug=True, num_devices=8)
    dp = nc.declare_dram_parameter
    xo_d = dp("xown", [P, 16 * H], F32, isOutput=False)
    vo_d = dp("vecown", [P, 16 * 3 * H], F16, isOutput=False)
    ei_d = dp("eidx", [P, NT], I32, isOutput=False)
    ev_d = dp("evp", [P, 3 * NT], F32, isOutput=False)
    dw_d = dp("dstw", [P, NT], F32, isOutput=False)
    rb_d = dp("rbtW", [65, NT * P], F16, isOutput=False)
    wpk_d = dp("wpk", [P, CW], F16, isOutput=False)
    bpk_d = dp("bpk", [P, CB], F32, isOutput=False)
    out_d = dp("outT", [4, 2048], F32, isOutput=True)

    with tile.TileContext(nc) as tc:
        with tc.tile_pool(name="persist", bufs=1) as PR, \
             tc.tile_pool(name="dpool", bufs=1, space="DRAM") as DP:
            wpk = PR.tile([P, CW], F16, tag="wpk", name="wpk")
            nc.sync.dma_start(out=wpk[:], in_=wpk_d[:, :])
            bpk = PR.tile([P, CB], F32, tag="bpk", name="bpk")
            nc.sync.dma_start(out=bpk[:], in_=bpk_d[:, :])
            xot = PR.tile([P, 16 * H], F32, tag="xot", name="xot")
            nc.sync.dma_start(out=xot[:], in_=xo_d[:, :])
            vot = PR.tile([P, 16 * 3 * H], F16, tag="vot", name="vot")
            nc.scalar.dma_start(out=vot[:], in_=vo_d[:, :])

            def W(nm):
                c0, r, w = WOFF[nm]
                return wpk[0:r, c0:c0 + w]

            def B(nm):
                j = BLAYOUT.index(nm)
                return bpk[:, j:j + 1]

            wx1 = [W("wx1_0"), W("wx1_1")]
            bx1 = [B("bx1_0"), B("bx1_1")]
            wx2 = [W("wx2_0"), W("wx2_1")]
            wrbf = W("wrbf")
            wvp = [W("wvp_0"), W("wvp_1")]
            wxv1 = [W(f"wxv1_{k}") for k in range(4)]
            bxv1 = [B("bxv1_0"), B("bxv1_1")]
            wxv2 = [W("wxv2_0"), W("wxv2_1")]
            o1wv1 = [W("o1wv1_0"), W("o1wv1_1")]
            o1wv2 = [W("o1wv2_0"), W("o1wv2_1")]
            o1wu1 = [W(f"o1wu1_{k}") for k in range(4)]
            o1bu1 = [B("o1bu1_0"), B("o1bu1_1")]
            o1wu2 = [W("o1wu2_0"), W("o1wu2_1")]
            o2wv1a = W("o2wv1a")
            o2wu1 = [W("o2wu1_0"), W("o2wu1_1")]
            o2bu1 = B("o2bu1")
            o2wu2c = W("o2wu2c")
            iotaF = W("iotaF")
            eidx = PR.tile([P, NT], I32, tag="eidx", name="eidx")
            nc.scalar.dma_start(out=eidx[:], in_=ei_d[:, :])
            evp = PR.tile([P, 3 * NT], F32, tag="evp", name="evp")
            nc.scalar.dma_start(out=evp[:], in_=ev_d[:, :])
            dstw = PR.tile([P, NT], F32, tag="dstw", name="dstw")
            nc.scalar.dma_start(out=dstw[:], in_=dw_d[:, :])
            eps5 = PR.tile([P, 1], F32, tag="eps5", name="eps5")
            nc.vector.memset(eps5[:], 1e-5)
            eps8 = PR.tile([P, 1], F32, tag="eps8", name="eps8")
            nc.vector.memset(eps8[:], 1e-8)

            CH_START = CH
            gown = [DP.tile([(CH_START[k + 1] - CH_START[k]) * P, GW], F16,
                            tag=f"gown{k}", name=f"gown{k}") for k in range(4)]
            gfull = nc.dram_tensor("gfull", [N, GW], F16, addr_space="Shared")

            def xo(i):
                return xot[:, i * H:(i + 1) * H]

            def vo(i):
                return vot[:, i * 3 * H:(i + 1) * 3 * H]

            def rep3(ap):
                return ap.rearrange("p (o f) -> p o f", o=1).broadcast_to([P, 3, H])

            # ---------------- phase 1: g-pack own nodes + AllGather ----------------
            with tc.tile_pool(name="p1", bufs=2) as S1, \
                 tc.tile_pool(name="q1", bufs=2, space="PSUM") as Q1:
                mvs = []
                for i in range(16):
                    st6 = S1.tile([P, 6], F32, tag="st6", bufs=3)
                    nc.vector.bn_stats(out=st6[:], in_=xo(i))
                    mv = S1.tile([P, 2], F32, tag=f"mv{i}", name=f"mv{i}")
                    nc.vector.bn_aggr(out=mv[:], in_=st6[:])
                    mvs.append(mv)
                sds = []
                for i in range(16):
                    sd = S1.tile([P, 1], F32, tag=f"sd{i}")
                    nc.scalar.activation(out=sd[:], in_=mvs[i][:, 1:2], func=FN.Sqrt,
                                         bias=eps5[:, 0:1])
                    sds.append(sd)
                for i in range(16):
                    ri = S1.tile([P, 1], F32, tag="ri", bufs=3)
                    nc.vector.reciprocal(out=ri[:], in_=sds[i][:])
                    xh = S1.tile([P, H], F16, tag="xh", bufs=4)
                    nc.vector.tensor_scalar(out=xh[:], in0=xo(i), scalar1=mvs[i][:, 0:1],
                                            scalar2=ri[:, 0:1], op0=A.subtract, op1=A.mult)
                    xhT = S1.tile([P, H], F16, tag="xhT", bufs=4)
                    for k in range(2):
                        (nc.sync if k == 0 else nc.scalar).dma_start_transpose(
                            out=xhT[:, k * P:(k + 1) * P], in_=xh[:, k * P:(k + 1) * P])
                    hT = Q1.tile([P, H], F32, tag="p1h", bufs=3)
                    for mo in range(2):
                        for k in range(2):
                            nc.tensor.matmul(out=hT[:, mo * P:(mo + 1) * P],
                                             lhsT=wx1[k][:, mo * P:(mo + 1) * P],
                                             rhs=xhT[:, k * P:(k + 1) * P],
                                             start=(k == 0), stop=(k == 1))
                    silT = S1.tile([P, H], F16, tag="silT", bufs=4)
                    for mo in range(2):
                        nc.scalar.activation(out=silT[:, mo * P:(mo + 1) * P],
                                             in_=hT[:, mo * P:(mo + 1) * P],
                                             func=FN.Silu, bias=bx1[mo][:, 0:1])
                    pA = Q1.tile([P, 2 * H], F32, tag="p1a", bufs=3)
                    pB = Q1.tile([P, H], F32, tag="p1b", bufs=2)
                    for mo in range(2):
                        nc.tensor.matmul(out=pA[:], lhsT=silT[:, mo * P:(mo + 1) * P],
                                         rhs=wx2[mo][:, 0:2 * H], start=(mo == 0), stop=(mo == 1))
                        nc.tensor.matmul(out=pB[:], lhsT=silT[:, mo * P:(mo + 1) * P],
                                         rhs=wx2[mo][:, 2 * H:3 * H], start=(mo == 0), stop=(mo == 1))
                    gp = S1.tile([P, GW], F16, tag="gp", bufs=4)
                    nc.scalar.activation(out=gp[:, 0:H], in_=pA[:, 0:H], func=FN.Copy)
                    nc.vector.tensor_tensor(
                        out=gp[:, H:4 * H].rearrange("p (c f) -> p c f", c=3),
                        in0=vo(i).rearrange("p (c f) -> p c f", c=3),
                        in1=rep3(pA[:, H:2 * H]), op=A.mult)
                    nc.scalar.activation(out=gp[:, 4 * H:5 * H], in_=pB[:, 0:H], func=FN.Copy)
                    k = next(kk for kk in range(4) if CH_START[kk] <= i < CH_START[kk + 1])
                    gi0 = i - CH_START[k]
                    nc.gpsimd.dma_start(out=gown[k][gi0 * P:(gi0 + 1) * P, :], in_=gp[:])
                    if i == CH_START[k + 1] - 1:
                        nc.gpsimd.collective_compute(
                            "AllGather", A.bypass, replica_groups=[list(range(8))],
                            ins=[gown[k][:].opt()],
                            outs=[gfull[CH_START[k] * 1024:CH_START[k + 1] * 1024, :].opt()])

            # -------- fused phase 2 (messages + scatter) / phase 3 (update + output) --------
            with tc.tile_pool(name="p2", bufs=2) as S2, \
                 tc.tile_pool(name="q2", bufs=2, space="PSUM") as Q2:
                for pr in range(8):
                    # pair-level transposed tiles: [128 feat, 256 nodes]
                    x1T = S2.tile([P, 2 * H], F16, tag="x1T", bufs=2)
                    v1T = [S2.tile([P, 2 * H], F16, tag=f"v1T{c}", bufs=2, name=f"v1T{c}_{pr}") for c in range(3)]
                    for wi in range(2):
                        w = 2 * pr + wi
                        winAB = Q2.tile([P, 4 * H], F32, tag="winAB", bufs=1)
                        rbtWt = S2.tile([65, T * P], F16, tag="rbtW", bufs=2)
                        nc.sync.dma_start(out=rbtWt[:], in_=rb_d[:, w * T * P:(w + 1) * T * P])
                        for tw in range(T):
                            t = w * T + tw
                            gt = S2.tile([P, GW], F16, tag="gt", bufs=8)
                            nc.gpsimd.indirect_dma_start(
                                out=gt[:], out_offset=None, in_=gfull[:],
                                in_offset=bass.IndirectOffsetOnAxis(ap=eidx[:, t:t + 1], axis=0))
                            stt = S2.tile([P, P], F16, tag="stt", bufs=4)
                            nc.vector.tensor_scalar(out=stt[:], in0=iotaF,
                                                    scalar1=dstw[:, t:t + 1], scalar2=None,
                                                    op0=A.is_equal)
                            rAx = Q2.tile([P, 4 * H], F32, tag="rAx", bufs=2)
                            nc.tensor.matmul(out=rAx[:, 0:512], lhsT=rbtWt[:, tw * P:(tw + 1) * P],
                                             rhs=wrbf[:, 0:512], start=True, stop=True)
                            nc.tensor.matmul(out=rAx[:, 512:1024], lhsT=rbtWt[:, tw * P:(tw + 1) * P],
                                             rhs=wrbf[:, 512:1024], start=True, stop=True)
                            rsb = S2.tile([P, 4 * H], F16, tag="rsb", bufs=3)
                            nc.scalar.activation(out=rsb[:], in_=rAx[:], func=FN.Copy)
                            mall = S2.tile([P, 7 * H], F16, tag="mall", bufs=3)
                            nc.vector.tensor_tensor(out=mall[:, 0:2 * H], in0=gt[:, H:3 * H],
                                                    in1=rsb[:, H:3 * H], op=A.mult)
                            nc.vector.tensor_tensor(out=mall[:, 2 * H:3 * H],
                                                    in0=gt[:, 3 * H:4 * H],
                                                    in1=rsb[:, H:2 * H], op=A.mult)
                            nc.vector.tensor_tensor(out=mall[:, 3 * H:4 * H],
                                                    in0=gt[:, 0:H],
                                                    in1=rsb[:, 0:H], op=A.mult)
                            nc.vector.tensor_tensor(out=mall[:, 4 * H:5 * H],
                                                    in0=gt[:, 4 * H:5 * H],
                                                    in1=rsb[:, 3 * H:4 * H], op=A.mult)
                            for c in (1, 2):
                                nc.vector.tensor_scalar(
                                    out=mall[:, (4 + c) * H:(5 + c) * H],
                                    in0=mall[:, 4 * H:5 * H],
                                    scalar1=evp[:, 3 * t + c:3 * t + c + 1], scalar2=None,
                                    op0=A.mult)
                            nc.vector.tensor_scalar(
                                out=mall[:, 4 * H:5 * H], in0=mall[:, 4 * H:5 * H],
                                scalar1=evp[:, 3 * t:3 * t + 1], scalar2=None,
                                op0=A.mult)
                            st0, sp0 = (tw == 0), (tw == T - 1)
                            # m3c scatter first: start=True clears each bank once (tw 0)
                            nc.tensor.matmul(out=winAB[:, 0:512], lhsT=stt[:],
                                             rhs=mall[:, 1024:1536], start=st0, stop=False)
                            nc.tensor.matmul(out=winAB[:, 512:768], lhsT=stt[:],
                                             rhs=mall[:, 1536:1792], start=st0, stop=False)
                            # mv/m1 scatter accumulates into the same columns
                            nc.tensor.matmul(out=winAB[:, 0:512], lhsT=stt[:],
                                             rhs=mall[:, 0:512], start=False, stop=sp0)
                            nc.tensor.matmul(out=winAB[:, 512:1024], lhsT=stt[:],
                                             rhs=mall[:, 512:1024], start=False, stop=sp0)
                        x1 = S2.tile([P, H], F16, tag="x1", bufs=2)
                        nc.vector.tensor_tensor(out=x1[:], in0=xo(w), in1=winAB[:, 3 * H:4 * H],
                                                op=A.add)
                        v1 = S2.tile([P, 3 * H], F16, tag="v1", bufs=2)
                        nc.vector.tensor_tensor(out=v1[:], in0=vo(w), in1=winAB[:, 0:3 * H],
                                                op=A.add)
                        for k in range(2):
                            nc.sync.dma_start_transpose(
                                out=x1T[:, k * 2 * P + wi * P:k * 2 * P + (wi + 1) * P],
                                in_=x1[:, k * P:(k + 1) * P])
                        for c in range(3):
                            eng = nc.sync
                            for k in range(2):
                                eng.dma_start_transpose(
                                    out=v1T[c][:, k * 2 * P + wi * P:k * 2 * P + (wi + 1) * P],
                                    in_=v1[:, c * H + k * P:c * H + (k + 1) * P])

                    # ---------------- phase 3 on the pair (256 node columns) ----------------
                    n2 = 2 * P
                    ppn = [0]
                    def pp(cols):
                        ppn[0] += 1
                        return Q2.tile([P, cols], F32, tag="pp", bufs=2,
                                       name=f"pp_{pr}_{ppn[0]}")
                    # vp = vec @ Wvp (transposed out), vd, vnorm
                    vp1sb = []
                    sqs = []
                    prods = []
                    for c in range(3):
                        vp1_ps = pp(2 * H)
                        vp2_ps = pp(2 * H)
                        for m in range(4):
                            dst_ps = vp1_ps if m < 2 else vp2_ps
                            mm = m % 2
                            for k in range(2):
                                nc.tensor.matmul(
                                    out=dst_ps[:, mm * n2:(mm + 1) * n2],
                                    lhsT=wvp[k][:, m * P:(m + 1) * P],
                                    rhs=v1T[c][:, k * n2:(k + 1) * n2],
                                    start=(k == 0), stop=(k == 1))
                        v1sb = S2.tile([P, 2 * H], F16, tag=f"vp1sb{c}", bufs=2)
                        nc.scalar.activation(out=v1sb[:], in_=vp1_ps[:], func=FN.Copy)
                        vp1sb.append(v1sb)
                        prod = S2.tile([P, 2 * H], F16, tag=f"prod{c}", bufs=1)
                        nc.vector.tensor_tensor(out=prod[:], in0=v1sb[:], in1=vp2_ps[:],
                                                op=A.mult)
                        prods.append(prod)
                        sq = S2.tile([P, 2 * H], F32, tag=f"sq{c}", bufs=1)
                        nc.scalar.activation(out=sq[:], in_=vp2_ps[:], func=FN.Square)
                        sqs.append(sq)
                    vns = S2.tile([P, 2 * H], F32, tag="vns", bufs=1)
                    nc.vector.tensor_tensor(out=vns[:], in0=sqs[0][:], in1=sqs[1][:], op=A.add)
                    nc.vector.tensor_tensor(out=vns[:], in0=vns[:], in1=sqs[2][:], op=A.add)
                    vnT = S2.tile([P, 2 * H], F16, tag="vnT", bufs=2)
                    nc.scalar.activation(out=vnT[:], in_=vns[:], func=FN.Sqrt, bias=eps8[:, 0:1])
                    # MLP-U: h = silu([x1|vn] @ Wxv1 + b)
                    hT_ps = pp(2 * H)
                    ins4 = [x1T, x1T, vnT, vnT]
                    for mo in range(2):
                        for kk in range(4):
                            nc.tensor.matmul(
                                out=hT_ps[:, mo * n2:(mo + 1) * n2],
                                lhsT=wxv1[kk][:, mo * P:(mo + 1) * P],
                                rhs=ins4[kk][:, (kk % 2) * n2:(kk % 2 + 1) * n2],
                                start=(kk == 0), stop=(kk == 3))
                    silT2 = S2.tile([P, 2 * H], F16, tag="silT2", bufs=2)
                    for mo in range(2):
                        nc.scalar.activation(out=silT2[:, mo * n2:(mo + 1) * n2],
                                             in_=hT_ps[:, mo * n2:(mo + 1) * n2],
                                             func=FN.Silu, bias=bxv1[mo][:, 0:1])
                    # xvh = silu @ Wxv2  (xv1: chunks 0-1, xv2: 2-3, xv3: 4-5)
                    xv1_ps = pp(2 * H)
                    xv2_ps = pp(2 * H)
                    xv3_ps = pp(2 * H)
                    for j in range(6):
                        dst_ps = (xv1_ps, xv2_ps, xv3_ps)[j // 2]
                        out_ap = dst_ps[:, (j % 2) * n2:(j % 2 + 1) * n2]
                        for mo in range(2):
                            nc.tensor.matmul(out=out_ap,
                                             lhsT=wxv2[mo][:, j * P:(j + 1) * P],
                                             rhs=silT2[:, mo * n2:(mo + 1) * n2],
                                             start=(mo == 0), stop=(mo == 1))
                    # vdT = sum_c vp1T_c * vp2T_c   (per node/feature, channels summed)
                    vdt = S2.tile([P, 2 * H], F16, tag="vdt", bufs=1)
                    nc.vector.tensor_tensor(out=vdt[:], in0=prods[0][:], in1=prods[1][:],
                                            op=A.add)
                    vdT = S2.tile([P, 2 * H], F16, tag="vdT", bufs=2)
                    nc.vector.tensor_tensor(out=vdT[:], in0=vdt[:], in1=prods[2][:], op=A.add)
                    # x2 = x1 + xv1 + xv2*vd
                    xv2sb = S2.tile([P, 2 * H], F16, tag="xv2sb", bufs=2)
                    nc.scalar.activation(out=xv2sb[:], in_=xv2_ps[:], func=FN.Copy)
                    tg = S2.tile([P, 2 * H], F16, tag="tg", bufs=2)
                    nc.vector.tensor_tensor(out=tg[:], in0=xv2sb[:], in1=vdT[:], op=A.mult)
                    ux = S2.tile([P, 2 * H], F16, tag="ux", bufs=2)
                    nc.vector.tensor_tensor(out=ux[:], in0=x1T[:], in1=xv1_ps[:], op=A.add)
                    x2T = S2.tile([P, 2 * H], F16, tag="x2T", bufs=2)
                    nc.vector.tensor_tensor(out=x2T[:], in0=ux[:], in1=tg[:], op=A.add)
                    # vec2_c = v1_c + xv3 * vp1_c
                    vec2T = []
                    for c in range(3):
                        s = S2.tile([P, 2 * H], F16, tag="s3", bufs=3)
                        nc.vector.tensor_tensor(out=s[:], in0=vp1sb[c][:], in1=xv3_ps[:],
                                                op=A.mult)
                        v2 = S2.tile([P, 2 * H], F16, tag=f"vec2T{c}", bufs=2)
                        nc.vector.tensor_tensor(out=v2[:], in0=v1T[c][:], in1=s[:], op=A.add)
                        vec2T.append(v2)
                    # ---- gated block 1 ----
                    sq1s = []
                    for c in range(3):
                        pw1_ps = pp(2 * H)
                        for m in range(2):
                            for k in range(2):
                                nc.tensor.matmul(out=pw1_ps[:, m * n2:(m + 1) * n2],
                                                 lhsT=o1wv1[k][:, m * P:(m + 1) * P],
                                                 rhs=vec2T[c][:, k * n2:(k + 1) * n2],
                                                 start=(k == 0), stop=(k == 1))
                        sq1 = S2.tile([P, 2 * H], F32, tag=f"sq1_{c}", bufs=1)
                        nc.scalar.activation(out=sq1[:], in_=pw1_ps[:], func=FN.Square)
                        sq1s.append(sq1)
                    vns1 = S2.tile([P, 2 * H], F32, tag="vns1", bufs=1)
                    nc.vector.tensor_tensor(out=vns1[:], in0=sq1s[0][:], in1=sq1s[1][:], op=A.add)
                    nc.vector.tensor_tensor(out=vns1[:], in0=vns1[:], in1=sq1s[2][:], op=A.add)
                    vn1T = S2.tile([P, 2 * H], F16, tag="vn1T", bufs=2)
                    nc.scalar.activation(out=vn1T[:], in_=vns1[:], func=FN.Sqrt)
                    hT1_ps = pp(2 * H)
                    ins1 = [x2T, x2T, vn1T, vn1T]
                    for mo in range(2):
                        for kk in range(4):
                            nc.tensor.matmul(
                                out=hT1_ps[:, mo * n2:(mo + 1) * n2],
                                lhsT=o1wu1[kk][:, mo * P:(mo + 1) * P],
                                rhs=ins1[kk][:, (kk % 2) * n2:(kk % 2 + 1) * n2],
                                start=(kk == 0), stop=(kk == 3))
                    sil1T = S2.tile([P, 2 * H], F16, tag="sil1T", bufs=2)
                    for mo in range(2):
                        nc.scalar.activation(out=sil1T[:, mo * n2:(mo + 1) * n2],
                                             in_=hT1_ps[:, mo * n2:(mo + 1) * n2],
                                             func=FN.Silu, bias=o1bu1[mo][:, 0:1])
                    ph1_ps = pp(2 * H)
                    for j in range(2):
                        for mo in range(2):
                            nc.tensor.matmul(out=ph1_ps[:, j * n2:(j + 1) * n2],
                                             lhsT=o1wu2[mo][:, j * P:(j + 1) * P],
                                             rhs=sil1T[:, mo * n2:(mo + 1) * n2],
                                             start=(mo == 0), stop=(mo == 1))
                    xnT = S2.tile([P, n2], F16, tag="xnT", bufs=2)
                    nc.scalar.activation(out=xnT[:], in_=ph1_ps[:, 0:n2], func=FN.Silu)
                    gate = S2.tile([P, n2], F16, tag="gate", bufs=2)
                    nc.scalar.activation(out=gate[:], in_=ph1_ps[:, n2:2 * n2], func=FN.Copy)
                    vnbT = []
                    for c in range(3):
                        v2b_ps = pp(n2)
                        for k in range(2):
                            nc.tensor.matmul(out=v2b_ps[:],
                                             lhsT=o1wv2[k][:],
                                             rhs=vec2T[c][:, k * n2:(k + 1) * n2],
                                             start=(k == 0), stop=(k == 1))
                        vb = S2.tile([P, n2], F16, tag=f"vnbT{c}", bufs=2)
                        nc.vector.tensor_tensor(out=vb[:], in0=v2b_ps[:], in1=gate[:], op=A.mult)
                        vnbT.append(vb)
                    # ---- gated block 2 ----
                    v2fsb = S2.tile([1, 3 * n2], F16, tag="v2fsb", bufs=2)
                    sq2s = []
                    for c in range(3):
                        pw2_ps = pp(n2)
                        nc.tensor.matmul(out=pw2_ps[:], lhsT=o2wv1a[:, 0:P], rhs=vnbT[c][:],
                                         start=True, stop=True)
                        v2f_ps = pp(n2)[0:1, :]
                        nc.tensor.matmul(out=v2f_ps, lhsT=o2wv1a[:, P:P + 1], rhs=vnbT[c][:],
                                         start=True, stop=True)
                        nc.scalar.activation(out=v2fsb[0:1, c * n2:(c + 1) * n2], in_=v2f_ps,
                                             func=FN.Copy)
                        sq2 = S2.tile([P, n2], F32, tag=f"sq2_{c}", bufs=1)
                        nc.scalar.activation(out=sq2[:], in_=pw2_ps[:], func=FN.Square)
                        sq2s.append(sq2)
                    vns2 = S2.tile([P, n2], F32, tag="vns2", bufs=1)
                    nc.vector.tensor_tensor(out=vns2[:], in0=sq2s[0][:], in1=sq2s[1][:], op=A.add)
                    nc.vector.tensor_tensor(out=vns2[:], in0=vns2[:], in1=sq2s[2][:], op=A.add)
                    vn2T = S2.tile([P, n2], F16, tag="vn2T", bufs=2)
                    nc.scalar.activation(out=vn2T[:], in_=vns2[:], func=FN.Sqrt)
                    h2_ps = pp(n2)
                    nc.tensor.matmul(out=h2_ps[:], lhsT=o2wu1[0][:], rhs=xnT[:],
                                     start=True, stop=False)
                    nc.tensor.matmul(out=h2_ps[:], lhsT=o2wu1[1][:], rhs=vn2T[:],
                                     start=False, stop=True)
                    sil2T = S2.tile([P, n2], F16, tag="sil2T", bufs=2)
                    nc.scalar.activation(out=sil2T[:], in_=h2_ps[:], func=FN.Silu,
                                         bias=o2bu1[:, 0:1])
                    phb_ps = pp(n2)[0:1, :]
                    nc.tensor.matmul(out=phb_ps, lhsT=o2wu2c[:], rhs=sil2T[:],
                                     start=True, stop=True)
                    hbs = S2.tile([1, n2], F16, tag="hbs", bufs=2)
                    nc.scalar.activation(out=hbs[:], in_=phb_ps, func=FN.Copy)
                    otp = S2.tile([P, n2], F32, tag="otp", bufs=2)
                    for c in range(3):
                        nc.vector.tensor_tensor(out=otp[32 * c:32 * c + 1, :],
                                                in0=v2fsb[0:1, c * n2:(c + 1) * n2],
                                                in1=hbs[0:1, :], op=A.mult)
                        nc.sync.dma_start(out=out_d[c:c + 1, pr * n2:(pr + 1) * n2],
                                          in_=otp[32 * c:32 * c + 1, :])

    nc.compile()
    return nc


def _make_inputs(inputs):
    f = _fold(inputs)
    T, NT, eidx, evp, dstw, rbtW = _pack(
        inputs["edge_index"], inputs["edge_rbf"], inputs["edge_vector"])
    x = np.asarray(inputs["x"], np.float32)
    vecf = np.asarray(inputs["vec"], np.float32).reshape(N, 3 * H)
    ins = []
    for c in range(8):
        xs = x[c * 2048:(c + 1) * 2048].reshape(16, P, H).transpose(1, 0, 2).reshape(P, 16 * H)
        vs = vecf[c * 2048:(c + 1) * 2048].reshape(16, P, 3 * H).transpose(1, 0, 2).reshape(P, 16 * 3 * H).astype(F16NP)
        d = {
            "xown": np.ascontiguousarray(xs), "vecown": np.ascontiguousarray(vs),
            "eidx": eidx[c], "evp": evp[c], "dstw": dstw[c], "rbtW": rbtW[c],
        }
        d.update(f)
        ins.append(d)
    return T, NT, ins


def kernel(**inputs):
    T, NT, ins = _make_inputs(inputs)
    nc = _build(NT, T)
    res = run_bass_kernel_spmd(nc, ins, list(range(8)))
    out = np.concatenate([res.results[c]["outT"][:3].T for c in range(8)], axis=0)
    return np.ascontiguousarray(out.astype(np.float32))


# revision 21
# speedup vs baseline: 1.1307x; 1.0075x over previous
import sys
sys.path.insert(0, '/opt/trn_rl_repo')
import numpy as np
import concourse.bass as bass
import concourse.bacc as bacc
import concourse.mybir as mybir
import concourse.tile as tile
from concourse.bass_utils import run_bass_kernel_spmd

N, E0, H = 16384, 262144, 256
P = 128
CH = [0, 2, 6, 11, 16]
GW = 5 * 256
F16NP = np.float16
F16 = mybir.dt.float16
F32 = mybir.dt.float32
BF16 = mybir.dt.bfloat16
F8 = mybir.dt.float8e4
I32 = mybir.dt.int32
S3 = 1.0 / np.sqrt(3.0)
SH = 1.0 / 16.0
S2 = 1.0 / np.sqrt(2.0)
INV06 = 1.0 / 0.6

# packed fp16 weight layout: (name, rows, cols)
WLAYOUT = [
    ("wx1_0", 128, 256), ("wx1_1", 128, 256),
    ("wx2_0", 128, 768), ("wx2_1", 128, 768),
    ("wrbf", 65, 1024),
    ("wvp_0", 128, 512), ("wvp_1", 128, 512),
    ("wxv1_0", 128, 256), ("wxv1_1", 128, 256), ("wxv1_2", 128, 256), ("wxv1_3", 128, 256),
    ("wxv2_0", 128, 768), ("wxv2_1", 128, 768),
    ("o1wv1_0", 128, 256), ("o1wv1_1", 128, 256),
    ("o1wv2_0", 128, 128), ("o1wv2_1", 128, 128),
    ("o1wu1_0", 128, 256), ("o1wu1_1", 128, 256), ("o1wu1_2", 128, 256), ("o1wu1_3", 128, 256),
    ("o1wu2_0", 128, 256), ("o1wu2_1", 128, 256),
    ("o2wv1a", 128, 129),
    ("o2wu1_0", 128, 128), ("o2wu1_1", 128, 128),
    ("o2wu2c", 128, 1),
    ("iotaF", 128, 128),
]
WOFF = {}
_c = 0
for _nm, _r, _w in WLAYOUT:
    WOFF[_nm] = (_c, _r, _w)
    _c += _w
CW = _c
# f32 bias pack: col j per name
BLAYOUT = ["bx1_0", "bx1_1", "bxv1_0", "bxv1_1", "o1bu1_0", "o1bu1_1", "o2bu1"]
CB = len(BLAYOUT)


def _fold(inp):
    blk = {}
    ln_g = inp["ln_g"].astype(np.float64)
    ln_b = inp["ln_b"].astype(np.float64)
    Wx1 = inp["W_x1"].astype(np.float64)
    Wx1f = (ln_g[:, None] * Wx1).astype(np.float32).astype(F16NP)
    blk["wx1_0"], blk["wx1_1"] = Wx1f[:P], Wx1f[P:]
    bx1 = (inp["b_x1"] + (ln_b @ Wx1).astype(np.float32)).astype(np.float32)
    Wx2 = inp["W_x2"].astype(np.float64) * INV06
    Wx2[:, H:2 * H] *= S3 * SH
    Wx2[:, 2 * H:] *= SH
    Wx2f = Wx2.astype(np.float32).astype(F16NP)
    blk["wx2_0"], blk["wx2_1"] = Wx2f[:P], Wx2f[P:]
    Wr = np.vstack([inp["W_rbf"], inp["b_rbf"][None, :]]).astype(np.float32).astype(F16NP)
    blk["wrbf"] = np.concatenate([Wr[:, 0:256], Wr[:, 256:512],
                                  Wr[:, 256:512], Wr[:, 512:768]], axis=1)  # [r1|r2|r2|r3]
    Wvp = inp["W_vp"].astype(np.float64).copy()
    Wvp[:, :H] *= SH
    Wvpf = Wvp.astype(np.float32).astype(F16NP)
    blk["wvp_0"], blk["wvp_1"] = Wvpf[:P], Wvpf[P:]
    Wxv1f = inp["W_xv1"].astype(F16NP)
    for k in range(4):
        blk[f"wxv1_{k}"] = Wxv1f[k * P:(k + 1) * P]
    Wxv2 = inp["W_xv2"].astype(np.float64) * INV06
    Wxv2[:, :2 * H] *= S2
    Wxv2[:, 2 * H:] *= 16.0
    Wxv2f = Wxv2.astype(np.float32).astype(F16NP)
    blk["wxv2_0"], blk["wxv2_1"] = Wxv2f[:P], Wxv2f[P:]
    o1Wv1f = inp["o1_Wv1"].astype(F16NP)
    blk["o1wv1_0"], blk["o1wv1_1"] = o1Wv1f[:P], o1Wv1f[P:]
    o1Wv2f = inp["o1_Wv2"].astype(F16NP)
    blk["o1wv2_0"], blk["o1wv2_1"] = o1Wv2f[:P], o1Wv2f[P:]
    o1Wu1f = inp["o1_Wu1"].astype(F16NP)
    for k in range(4):
        blk[f"o1wu1_{k}"] = o1Wu1f[k * P:(k + 1) * P]
    o1Wu2f = (inp["o1_Wu2"].astype(np.float64) * INV06).astype(np.float32).astype(F16NP)
    blk["o1wu2_0"], blk["o1wu2_1"] = o1Wu2f[:P], o1Wu2f[P:]
    blk["o2wv1a"] = np.hstack([inp["o2_Wv1"], inp["o2_Wv2"]]).astype(F16NP)
    o2Wu1 = inp["o2_Wu1"].astype(np.float64).copy()
    o2Wu1[:P, :] *= INV06
    o2Wu1f = o2Wu1.astype(np.float32).astype(F16NP)
    blk["o2wu1_0"], blk["o2wu1_1"] = o2Wu1f[:P], o2Wu1f[P:]
    blk["o2wu2c"] = (inp["o2_Wu2"][:, 1:2].astype(np.float64) * INV06).astype(np.float32).astype(F16NP)
    blk["iotaF"] = np.broadcast_to(np.arange(P, dtype=F16NP)[None, :], (P, P)).copy()
    wpk = np.zeros((P, CW), F16NP)
    for nm, r, w in WLAYOUT:
        c0 = WOFF[nm][0]
        wpk[:r, c0:c0 + w] = blk[nm]
    bpk = np.zeros((P, CB), np.float32)
    bpk[:, 0] = bx1[:P]
    bpk[:, 1] = bx1[P:]
    bpk[:, 2] = inp["b_xv1"][:P]
    bpk[:, 3] = inp["b_xv1"][P:]
    bpk[:, 4] = inp["o1_bu1"][:P]
    bpk[:, 5] = inp["o1_bu1"][P:]
    bpk[:, 6] = inp["o2_bu1"][:P]
    for nm in ("b_x2", "o1_bu2", "o2_bu2"):
        assert not np.any(inp[nm]), f"nonzero {nm} unsupported by folding"
    return {"wpk": wpk, "bpk": bpk}


def _pack(edge_index, edge_rbf, edge_vector):
    E = edge_index.shape[1]
    src = edge_index[0].astype(np.int64)
    dst = edge_index[1].astype(np.int64)
    gw = dst >> 7
    order = np.argsort(gw, kind="stable")
    gs = gw[order]
    srcs = src[order]
    dsts = dst[order]
    rbfs = edge_rbf[order]
    evs = edge_vector[order]
    cnt = np.bincount(gs, minlength=P)
    T = int(np.ceil(cnt.max() / P))
    NT = 16 * T
    startw = np.concatenate([[0], np.cumsum(cnt)[:-1]])
    r = np.arange(E) - startw[gs]
    core = (gs >> 4).astype(np.int64)
    wl = gs & 15
    tw = r >> 7
    p = r & 127
    t = wl * T + tw
    # remap src node id -> row in the AllGather-ed gfull layout:
    # chunk k holds groups [CH_START[k]:CH_START[k+1]) of every core, core-major
    c_s = srcs >> 11
    j = srcs & 2047
    gi = j >> 7
    rr = j & 127
    CH_START = np.array(CH)
    CH_BASE = CH_START * 8 * 128
    kk = np.searchsorted(CH_START, gi, side="right") - 1
    glen = (CH_START[kk + 1] - CH_START[kk])
    gidx = (CH_BASE[kk] + c_s * glen * 128 + (gi - CH_START[kk]) * 128 + rr).astype(np.int32)
    eidx = np.zeros((8, P, NT), np.int32)
    evp = np.zeros((8, P, 3 * NT), np.float32)
    dstw = np.zeros((8, P, NT), np.float32)
    rbtW = np.zeros((8, 65, NT * P), F16NP)
    eidx[core, p, t] = gidx
    for c in range(3):
        evp[core, p, 3 * t + c] = evs[:, c]
    dstw[core, p, t] = (dsts & 127).astype(np.float32)
    cols = t * P + p
    rbtW[core[:, None], np.arange(64)[None, :], cols[:, None]] = rbfs.astype(F16NP)
    rbtW[core, 64, cols] = 1.0
    return T, NT, eidx, evp, dstw, rbtW


def _build(NT, T, dbg=False):
    A = mybir.AluOpType
    FN = mybir.ActivationFunctionType
    nc = bacc.Bacc("TRN2", target_bir_lowering=False, debug=True, num_devices=8)
    dp = nc.declare_dram_parameter
    xo_d = dp("xown", [P, 16 * H], F32, isOutput=False)
    vo_d = dp("vecown", [P, 16 * 3 * H], F16, isOutput=False)
    ei_d = dp("eidx", [P, NT], I32, isOutput=False)
    ev_d = dp("evp", [P, 3 * NT], F32, isOutput=False)
    dw_d = dp("dstw", [P, NT], F32, isOutput=False)
    rb_d = dp("rbtW", [65, NT * P], F16, isOutput=False)
    wpk_d = dp("wpk", [P, CW], F16, isOutput=False)
    bpk_d = dp("bpk", [P, CB], F32, isOutput=False)
    out_d = dp("outT", [4, 2048], F32, isOutput=True)

    with tile.TileContext(nc) as tc:
        with tc.tile_pool(name="persist", bufs=1) as PR, \
             tc.tile_pool(name="dpool", bufs=1, space="DRAM") as DP:
            wpk = PR.tile([P, CW], F16, tag="wpk", name="wpk")
            nc.sync.dma_start(out=wpk[:], in_=wpk_d[:, :])
            bpk = PR.tile([P, CB], F32, tag="bpk", name="bpk")
            nc.sync.dma_start(out=bpk[:], in_=bpk_d[:, :])
            xot = PR.tile([P, 16 * H], F32, tag="xot", name="xot")
            nc.sync.dma_start(out=xot[:], in_=xo_d[:, :])
            vot = PR.tile([P, 16 * 3 * H], F16, tag="vot", name="vot")
            nc.scalar.dma_start(out=vot[:], in_=vo_d[:, :])

            def W(nm):
                c0, r, w = WOFF[nm]
                return wpk[0:r, c0:c0 + w]

            def B(nm):
                j = BLAYOUT.index(nm)
                return bpk[:, j:j + 1]

            wx1 = [W("wx1_0"), W("wx1_1")]
            bx1 = [B("bx1_0"), B("bx1_1")]
            wx2 = [W("wx2_0"), W("wx2_1")]
            wrbf = W("wrbf")
            wvp = [W("wvp_0"), W("wvp_1")]
            wxv1 = [W(f"wxv1_{k}") for k in range(4)]
            bxv1 = [B("bxv1_0"), B("bxv1_1")]
            wxv2 = [W("wxv2_0"), W("wxv2_1")]
            o1wv1 = [W("o1wv1_0"), W("o1wv1_1")]
            o1wv2 = [W("o1wv2_0"), W("o1wv2_1")]
            o1wu1 = [W(f"o1wu1_{k}") for k in range(4)]
            o1bu1 = [B("o1bu1_0"), B("o1bu1_1")]
            o1wu2 = [W("o1wu2_0"), W("o1wu2_1")]
            o2wv1a = W("o2wv1a")
            o2wu1 = [W("o2wu1_0"), W("o2wu1_1")]
            o2bu1 = B("o2bu1")
            o2wu2c = W("o2wu2c")
            iotaF = W("iotaF")
            eidx = PR.tile([P, NT], I32, tag="eidx", name="eidx")
            nc.scalar.dma_start(out=eidx[:], in_=ei_d[:, :])
            evp = PR.tile([P, 3 * NT], F32, tag="evp", name="evp")
            nc.scalar.dma_start(out=evp[:], in_=ev_d[:, :])
            dstw = PR.tile([P, NT], F32, tag="dstw", name="dstw")
            nc.scalar.dma_start(out=dstw[:], in_=dw_d[:, :])
            eps5 = PR.tile([P, 1], F32, tag="eps5", name="eps5")
            nc.vector.memset(eps5[:], 1e-5)
            eps8 = PR.tile([P, 1], F32, tag="eps8", name="eps8")
            nc.vector.memset(eps8[:], 1e-8)

            CH_START = CH
            gown = [DP.tile([(CH_START[k + 1] - CH_START[k]) * P, GW], F16,
                            tag=f"gown{k}", name=f"gown{k}") for k in range(4)]
            gfull = nc.dram_tensor("gfull", [N, GW], F16, addr_space="Shared")

            def xo(i):
                return xot[:, i * H:(i + 1) * H]

            def vo(i):
                return vot[:, i * 3 * H:(i + 1) * 3 * H]

            def rep3(ap):
                return ap.rearrange("p (o f) -> p o f", o=1).broadcast_to([P, 3, H])

            # ---------------- phase 1: g-pack own nodes + AllGather ----------------
            with tc.tile_pool(name="p1", bufs=2) as S1, \
                 tc.tile_pool(name="q1", bufs=2, space="PSUM") as Q1:
                mvs = []
                for i in range(16):
                    st6 = S1.tile([P, 6], F32, tag="st6", bufs=3)
                    nc.vector.bn_stats(out=st6[:], in_=xo(i))
                    mv = S1.tile([P, 2], F32, tag=f"mv{i}", name=f"mv{i}")
                    nc.vector.bn_aggr(out=mv[:], in_=st6[:])
                    mvs.append(mv)
                sds = []
                for i in range(16):
                    sd = S1.tile([P, 1], F32, tag=f"sd{i}")
                    nc.scalar.activation(out=sd[:], in_=mvs[i][:, 1:2], func=FN.Sqrt,
                                         bias=eps5[:, 0:1])
                    sds.append(sd)
                for i in range(16):
                    ri = S1.tile([P, 1], F32, tag="ri", bufs=3)
                    nc.vector.reciprocal(out=ri[:], in_=sds[i][:])
                    xh = S1.tile([P, H], F16, tag="xh", bufs=4)
                    nc.vector.tensor_scalar(out=xh[:], in0=xo(i), scalar1=mvs[i][:, 0:1],
                                            scalar2=ri[:, 0:1], op0=A.subtract, op1=A.mult)
                    xhT = S1.tile([P, H], F16, tag="xhT", bufs=4)
                    for k in range(2):
                        (nc.sync if k == 0 else nc.scalar).dma_start_transpose(
                            out=xhT[:, k * P:(k + 1) * P], in_=xh[:, k * P:(k + 1) * P])
                    hT = Q1.tile([P, H], F32, tag="p1h", bufs=3)
                    for mo in range(2):
                        for k in range(2):
                            nc.tensor.matmul(out=hT[:, mo * P:(mo + 1) * P],
                                             lhsT=wx1[k][:, mo * P:(mo + 1) * P],
                                             rhs=xhT[:, k * P:(k + 1) * P],
                                             start=(k == 0), stop=(k == 1))
                    silT = S1.tile([P, H], F16, tag="silT", bufs=4)
                    for mo in range(2):
                        nc.scalar.activation(out=silT[:, mo * P:(mo + 1) * P],
                                             in_=hT[:, mo * P:(mo + 1) * P],
                                             func=FN.Silu, bias=bx1[mo][:, 0:1])
                    pA = Q1.tile([P, 2 * H], F32, tag="p1a", bufs=3)
                    pB = Q1.tile([P, H], F32, tag="p1b", bufs=2)
                    for mo in range(2):
                        nc.tensor.matmul(out=pA[:], lhsT=silT[:, mo * P:(mo + 1) * P],
                                         rhs=wx2[mo][:, 0:2 * H], start=(mo == 0), stop=(mo == 1))
                        nc.tensor.matmul(out=pB[:], lhsT=silT[:, mo * P:(mo + 1) * P],
                                         rhs=wx2[mo][:, 2 * H:3 * H], start=(mo == 0), stop=(mo == 1))
                    gp = S1.tile([P, GW], F16, tag="gp", bufs=4)
                    nc.scalar.activation(out=gp[:, 0:H], in_=pA[:, 0:H], func=FN.Copy)
                    nc.vector.tensor_tensor(
                        out=gp[:, H:4 * H].rearrange("p (c f) -> p c f", c=3),
                        in0=vo(i).rearrange("p (c f) -> p c f", c=3),
                        in1=rep3(pA[:, H:2 * H]), op=A.mult)
                    nc.scalar.activation(out=gp[:, 4 * H:5 * H], in_=pB[:, 0:H], func=FN.Copy)
                    k = next(kk for kk in range(4) if CH_START[kk] <= i < CH_START[kk + 1])
                    gi0 = i - CH_START[k]
                    nc.gpsimd.dma_start(out=gown[k][gi0 * P:(gi0 + 1) * P, :], in_=gp[:])
                    if i == CH_START[k + 1] - 1:
                        nc.gpsimd.collective_compute(
                            "AllGather", A.bypass, replica_groups=[list(range(8))],
                            ins=[gown[k][:].opt()],
                            outs=[gfull[CH_START[k] * 1024:CH_START[k + 1] * 1024, :].opt()])

            # -------- fused phase 2 (messages + scatter) / phase 3 (update + output) --------
            with tc.tile_pool(name="p2", bufs=2) as S2, \
                 tc.tile_pool(name="q2", bufs=2, space="PSUM") as Q2:
                for pr in range(8):
                    # pair-level transposed tiles: [128 feat, 256 nodes]
                    x1T = S2.tile([P, 2 * H], F16, tag="x1T", bufs=2)
                    v1T = [S2.tile([P, 2 * H], F16, tag=f"v1T{c}", bufs=2, name=f"v1T{c}_{pr}") for c in range(3)]
                    for wi in range(2):
                        w = 2 * pr + wi
                        winAB = Q2.tile([P, 4 * H], F32, tag="winAB", bufs=1)
                        rbtWt = S2.tile([65, T * P], F16, tag="rbtW", bufs=2)
                        nc.sync.dma_start(out=rbtWt[:], in_=rb_d[:, w * T * P:(w + 1) * T * P])
                        for tw in range(T):
                            t = w * T + tw
                            gt = S2.tile([P, GW], F16, tag="gt", bufs=10)
                            nc.gpsimd.indirect_dma_start(
                                out=gt[:], out_offset=None, in_=gfull[:],
                                in_offset=bass.IndirectOffsetOnAxis(ap=eidx[:, t:t + 1], axis=0))
                            stt = S2.tile([P, P], F16, tag="stt", bufs=4)
                            nc.vector.tensor_scalar(out=stt[:], in0=iotaF,
                                                    scalar1=dstw[:, t:t + 1], scalar2=None,
                                                    op0=A.is_equal)
                            rAx = Q2.tile([P, 4 * H], F32, tag="rAx", bufs=2)
                            nc.tensor.matmul(out=rAx[:, 0:512], lhsT=rbtWt[:, tw * P:(tw + 1) * P],
                                             rhs=wrbf[:, 0:512], start=True, stop=True)
                            nc.tensor.matmul(out=rAx[:, 512:1024], lhsT=rbtWt[:, tw * P:(tw + 1) * P],
                                             rhs=wrbf[:, 512:1024], start=True, stop=True)
                            rsb = S2.tile([P, 4 * H], F16, tag="rsb", bufs=3)
                            nc.scalar.activation(out=rsb[:], in_=rAx[:], func=FN.Copy)
                            mall = S2.tile([P, 7 * H], F16, tag="mall", bufs=4)
                            nc.vector.tensor_tensor(out=mall[:, 0:2 * H], in0=gt[:, H:3 * H],
                                                    in1=rsb[:, H:3 * H], op=A.mult)
                            nc.vector.tensor_tensor(out=mall[:, 2 * H:3 * H],
                                                    in0=gt[:, 3 * H:4 * H],
                                                    in1=rsb[:, H:2 * H], op=A.mult)
                            nc.vector.tensor_tensor(out=mall[:, 3 * H:4 * H],
                                                    in0=gt[:, 0:H],
                                                    in1=rsb[:, 0:H], op=A.mult)
                            nc.vector.tensor_tensor(out=mall[:, 4 * H:5 * H],
                                                    in0=gt[:, 4 * H:5 * H],
                                                    in1=rsb[:, 3 * H:4 * H], op=A.mult)
                            for c in (1, 2):
                                nc.vector.tensor_scalar(
                                    out=mall[:, (4 + c) * H:(5 + c) * H],
                                    in0=mall[:, 4 * H:5 * H],
                                    scalar1=evp[:, 3 * t + c:3 * t + c + 1], scalar2=None,
                                    op0=A.mult)
                            nc.vector.tensor_scalar(
                                out=mall[:, 4 * H:5 * H], in0=mall[:, 4 * H:5 * H],
                                scalar1=evp[:, 3 * t:3 * t + 1], scalar2=None,
                                op0=A.mult)
                            st0, sp0 = (tw == 0), (tw == T - 1)
                            # m3c scatter first: start=True clears each bank once (tw 0)
                            nc.tensor.matmul(out=winAB[:, 0:512], lhsT=stt[:],
                                             rhs=mall[:, 1024:1536], start=st0, stop=False)
                            nc.tensor.matmul(out=winAB[:, 512:768], lhsT=stt[:],
                                             rhs=mall[:, 1536:1792], start=st0, stop=False)
                            # mv/m1 scatter accumulates into the same columns
                            nc.tensor.matmul(out=winAB[:, 0:512], lhsT=stt[:],
                                             rhs=mall[:, 0:512], start=False, stop=sp0)
                            nc.tensor.matmul(out=winAB[:, 512:1024], lhsT=stt[:],
                                             rhs=mall[:, 512:1024], start=False, stop=sp0)
                        x1 = S2.tile([P, H], F16, tag="x1", bufs=2)
                        nc.vector.tensor_tensor(out=x1[:], in0=xo(w), in1=winAB[:, 3 * H:4 * H],
                                                op=A.add)
                        v1 = S2.tile([P, 3 * H], F16, tag="v1", bufs=2)
                        nc.vector.tensor_tensor(out=v1[:], in0=vo(w), in1=winAB[:, 0:3 * H],
                                                op=A.add)
                        for k in range(2):
                            nc.sync.dma_start_transpose(
                                out=x1T[:, k * 2 * P + wi * P:k * 2 * P + (wi + 1) * P],
                                in_=x1[:, k * P:(k + 1) * P])
                        for c in range(3):
                            eng = nc.sync
                            for k in range(2):
                                eng.dma_start_transpose(
                                    out=v1T[c][:, k * 2 * P + wi * P:k * 2 * P + (wi + 1) * P],
                                    in_=v1[:, c * H + k * P:c * H + (k + 1) * P])

                    # ---------------- phase 3 on the pair (256 node columns) ----------------
                    n2 = 2 * P
                    ppn = [0]
                    def pp(cols):
                        ppn[0] += 1
                        return Q2.tile([P, cols], F32, tag="pp", bufs=2,
                                       name=f"pp_{pr}_{ppn[0]}")
                    # vp = vec @ Wvp (transposed out), vd, vnorm
                    vp1sb = []
                    sqs = []
                    prods = []
                    for c in range(3):
                        vp1_ps = pp(2 * H)
                        vp2_ps = pp(2 * H)
                        for m in range(4):
                            dst_ps = vp1_ps if m < 2 else vp2_ps
                            mm = m % 2
                            for k in range(2):
                                nc.tensor.matmul(
                                    out=dst_ps[:, mm * n2:(mm + 1) * n2],
                                    lhsT=wvp[k][:, m * P:(m + 1) * P],
                                    rhs=v1T[c][:, k * n2:(k + 1) * n2],
                                    start=(k == 0), stop=(k == 1))
                        v1sb = S2.tile([P, 2 * H], F16, tag=f"vp1sb{c}", bufs=2)
                        nc.scalar.activation(out=v1sb[:], in_=vp1_ps[:], func=FN.Copy)
                        vp1sb.append(v1sb)
                        prod = S2.tile([P, 2 * H], F16, tag=f"prod{c}", bufs=1)
                        nc.vector.tensor_tensor(out=prod[:], in0=v1sb[:], in1=vp2_ps[:],
                                                op=A.mult)
                        prods.append(prod)
                        sq = S2.tile([P, 2 * H], F32, tag=f"sq{c}", bufs=1)
                        nc.scalar.activation(out=sq[:], in_=vp2_ps[:], func=FN.Square)
                        sqs.append(sq)
                    vns = S2.tile([P, 2 * H], F32, tag="vns", bufs=1)
                    nc.vector.tensor_tensor(out=vns[:], in0=sqs[0][:], in1=sqs[1][:], op=A.add)
                    nc.vector.tensor_tensor(out=vns[:], in0=vns[:], in1=sqs[2][:], op=A.add)
                    vnT = S2.tile([P, 2 * H], F16, tag="vnT", bufs=2)
                    nc.scalar.activation(out=vnT[:], in_=vns[:], func=FN.Sqrt, bias=eps8[:, 0:1])
                    # MLP-U: h = silu([x1|vn] @ Wxv1 + b)
                    hT_ps = pp(2 * H)
                    ins4 = [x1T, x1T, vnT, vnT]
                    for mo in range(2):
                        for kk in range(4):
                            nc.tensor.matmul(
                                out=hT_ps[:, mo * n2:(mo + 1) * n2],
                                lhsT=wxv1[kk][:, mo * P:(mo + 1) * P],
                                rhs=ins4[kk][:, (kk % 2) * n2:(kk % 2 + 1) * n2],
                                start=(kk == 0), stop=(kk == 3))
                    silT2 = S2.tile([P, 2 * H], F16, tag="silT2", bufs=2)
                    for mo in range(2):
                        nc.scalar.activation(out=silT2[:, mo * n2:(mo + 1) * n2],
                                             in_=hT_ps[:, mo * n2:(mo + 1) * n2],
                                             func=FN.Silu, bias=bxv1[mo][:, 0:1])
                    # xvh = silu @ Wxv2  (xv1: chunks 0-1, xv2: 2-3, xv3: 4-5)
                    xv1_ps = pp(2 * H)
                    xv2_ps = pp(2 * H)
                    xv3_ps = pp(2 * H)
                    for j in range(6):
                        dst_ps = (xv1_ps, xv2_ps, xv3_ps)[j // 2]
                        out_ap = dst_ps[:, (j % 2) * n2:(j % 2 + 1) * n2]
                        for mo in range(2):
                            nc.tensor.matmul(out=out_ap,
                                             lhsT=wxv2[mo][:, j * P:(j + 1) * P],
                                             rhs=silT2[:, mo * n2:(mo + 1) * n2],
                                             start=(mo == 0), stop=(mo == 1))
                    # vdT = sum_c vp1T_c * vp2T_c   (per node/feature, channels summed)
                    vdt = S2.tile([P, 2 * H], F16, tag="vdt", bufs=1)
                    nc.vector.tensor_tensor(out=vdt[:], in0=prods[0][:], in1=prods[1][:],
                                            op=A.add)
                    vdT = S2.tile([P, 2 * H], F16, tag="vdT", bufs=2)
                    nc.vector.tensor_tensor(out=vdT[:], in0=vdt[:], in1=prods[2][:], op=A.add)
                    # x2 = x1 + xv1 + xv2*vd
                    xv2sb = S2.tile([P, 2 * H], F16, tag="xv2sb", bufs=2)
                    nc.scalar.activation(out=xv2sb[:], in_=xv2_ps[:], func=FN.Copy)
                    tg = S2.tile([P, 2 * H], F16, tag="tg", bufs=2)
                    nc.vector.tensor_tensor(out=tg[:], in0=xv2sb[:], in1=vdT[:], op=A.mult)
                    ux = S2.tile([P, 2 * H], F16, tag="ux", bufs=2)
                    nc.vector.tensor_tensor(out=ux[:], in0=x1T[:], in1=xv1_ps[:], op=A.add)
                    x2T = S2.tile([P, 2 * H], F16, tag="x2T", bufs=2)
                    nc.vector.tensor_tensor(out=x2T[:], in0=ux[:], in1=tg[:], op=A.add)
                    # vec2_c = v1_c + xv3 * vp1_c
                    vec2T = []
                    for c in range(3):
                        s = S2.tile([P, 2 * H], F16, tag="s3", bufs=3)
                        nc.vector.tensor_tensor(out=s[:], in0=vp1sb[c][:], in1=xv3_ps[:],
                                                op=A.mult)
                        v2 = S2.tile([P, 2 * H], F16, tag=f"vec2T{c}", bufs=2)
                        nc.vector.tensor_tensor(out=v2[:], in0=v1T[c][:], in1=s[:], op=A.add)
                        vec2T.append(v2)
                    # ---- gated block 1 ----
                    sq1s = []
                    for c in range(3):
                        pw1_ps = pp(2 * H)
                        for m in range(2):
                            for k in range(2):
                                nc.tensor.matmul(out=pw1_ps[:, m * n2:(m + 1) * n2],
                                                 lhsT=o1wv1[k][:, m * P:(m + 1) * P],
                                                 rhs=vec2T[c][:, k * n2:(k + 1) * n2],
                                                 start=(k == 0), stop=(k == 1))
                        sq1 = S2.tile([P, 2 * H], F32, tag=f"sq1_{c}", bufs=1)
                        nc.scalar.activation(out=sq1[:], in_=pw1_ps[:], func=FN.Square)
                        sq1s.append(sq1)
                    vns1 = S2.tile([P, 2 * H], F32, tag="vns1", bufs=1)
                    nc.vector.tensor_tensor(out=vns1[:], in0=sq1s[0][:], in1=sq1s[1][:], op=A.add)
                    nc.vector.tensor_tensor(out=vns1[:], in0=vns1[:], in1=sq1s[2][:], op=A.add)
                    vn1T = S2.tile([P, 2 * H], F16, tag="vn1T", bufs=2)
                    nc.scalar.activation(out=vn1T[:], in_=vns1[:], func=FN.Sqrt)
                    hT1_ps = pp(2 * H)
                    ins1 = [x2T, x2T, vn1T, vn1T]
                    for mo in range(2):
                        for kk in range(4):
                            nc.tensor.matmul(
                                out=hT1_ps[:, mo * n2:(mo + 1) * n2],
                                lhsT=o1wu1[kk][:, mo * P:(mo + 1) * P],
                                rhs=ins1[kk][:, (kk % 2) * n2:(kk % 2 + 1) * n2],
                                start=(kk == 0), stop=(kk == 3))
                    sil1T = S2.tile([P, 2 * H], F16, tag="sil1T", bufs=2)
                    for mo in range(2):
                        nc.scalar.activation(out=sil1T[:, mo * n2:(mo + 1) * n2],
                                             in_=hT1_ps[:, mo * n2:(mo + 1) * n2],
                                             func=FN.Silu, bias=o1bu1[mo][:, 0:1])
                    ph1_ps = pp(2 * H)
                    for j in range(2):
                        for mo in range(2):
                            nc.tensor.matmul(out=ph1_ps[:, j * n2:(j + 1) * n2],
                                             lhsT=o1wu2[mo][:, j * P:(j + 1) * P],
                                             rhs=sil1T[:, mo * n2:(mo + 1) * n2],
                                             start=(mo == 0), stop=(mo == 1))
                    xnT = S2.tile([P, n2], F16, tag="xnT", bufs=2)
                    nc.scalar.activation(out=xnT[:], in_=ph1_ps[:, 0:n2], func=FN.Silu)
                    gate = S2.tile([P, n2], F16, tag="gate", bufs=2)
                    nc.scalar.activation(out=gate[:], in_=ph1_ps[:, n2:2 * n2], func=FN.Copy)
                    vnbT = []
                    for c in range(3):
                        v2b_ps = pp(n2)
                        for k in range(2):
                            nc.tensor.matmul(out=v2b_ps[:],
                                             lhsT=o1wv2[k][:],
                                             rhs=vec2T[c][:, k * n2:(k + 1) * n2],
                                             start=(k == 0), stop=(k == 1))
                        vb = S2.tile([P, n2], F16, tag=f"vnbT{c}", bufs=2)
                        nc.vector.tensor_tensor(out=vb[:], in0=v2b_ps[:], in1=gate[:], op=A.mult)
                        vnbT.append(vb)
                    # ---- gated block 2 ----
                    v2fsb = S2.tile([1, 3 * n2], F16, tag="v2fsb", bufs=2)
                    sq2s = []
                    for c in range(3):
                        pw2_ps = pp(n2)
                        nc.tensor.matmul(out=pw2_ps[:], lhsT=o2wv1a[:, 0:P], rhs=vnbT[c][:],
                                         start=True, stop=True)
                        v2f_ps = pp(n2)[0:1, :]
                        nc.tensor.matmul(out=v2f_ps, lhsT=o2wv1a[:, P:P + 1], rhs=vnbT[c][:],
                                         start=True, stop=True)
                        nc.scalar.activation(out=v2fsb[0:1, c * n2:(c + 1) * n2], in_=v2f_ps,
                                             func=FN.Copy)
                        sq2 = S2.tile([P, n2], F32, tag=f"sq2_{c}", bufs=1)
                        nc.scalar.activation(out=sq2[:], in_=pw2_ps[:], func=FN.Square)
                        sq2s.append(sq2)
                    vns2 = S2.tile([P, n2], F32, tag="vns2", bufs=1)
                    nc.vector.tensor_tensor(out=vns2[:], in0=sq2s[0][:], in1=sq2s[1][:], op=A.add)
                    nc.vector.tensor_tensor(out=vns2[:], in0=vns2[:], in1=sq2s[2][:], op=A.add)
                    vn2T = S2.tile([P, n2], F16, tag="vn2T", bufs=2)
                    nc.scalar.activation(out=vn2T[:], in_=vns2[:], func=FN.Sqrt)
                    h2_ps = pp(n2)
                    nc.tensor.matmul(out=h2_ps[:], lhsT=o2wu1[0][:], rhs=xnT[:],
                                     start=True, stop=False)
                    nc.tensor.matmul(out=h2_ps[:], lhsT=o2wu1[1][:], rhs=vn2T[:],
                                     start=False, stop=True)
                    sil2T = S2.tile([P, n2], F16, tag="sil2T", bufs=2)
                    nc.scalar.activation(out=sil2T[:], in_=h2_ps[:], func=FN.Silu,
                                         bias=o2bu1[:, 0:1])
                    phb_ps = pp(n2)[0:1, :]
                    nc.tensor.matmul(out=phb_ps, lhsT=o2wu2c[:], rhs=sil2T[:],
                                     start=True, stop=True)
                    hbs = S2.tile([1, n2], F16, tag="hbs", bufs=2)
                    nc.scalar.activation(out=hbs[:], in_=phb_ps, func=FN.Copy)
                    otp = S2.tile([P, n2], F32, tag="otp", bufs=2)
                    for c in range(3):
                        nc.vector.tensor_tensor(out=otp[32 * c:32 * c + 1, :],
                                                in0=v2fsb[0:1, c * n2:(c + 1) * n2],
                                                in1=hbs[0:1, :], op=A.mult)
                        nc.sync.dma_start(out=out_d[c:c + 1, pr * n2:(pr + 1) * n2],
                                          in_=otp[32 * c:32 * c + 1, :])

    nc.compile()
    return nc


def _make_inputs(inputs):
    f = _fold(inputs)
    T, NT, eidx, evp, dstw, rbtW = _pack(
        inputs["edge_index"], inputs["edge_rbf"], inputs["edge_vector"])
    x = np.asarray(inputs["x"], np.float32)
    vecf = np.asarray(inputs["vec"], np.float32).reshape(N, 3 * H)
    ins = []
    for c in range(8):
        xs = x[c * 2048:(c + 1) * 2048].reshape(16, P, H).transpose(1, 0, 2).reshape(P, 16 * H)
        vs = vecf[c * 2048:(c + 1) * 2048].reshape(16, P, 3 * H).transpose(1, 0, 2).reshape(P, 16 * 3 * H).astype(F16NP)
        d = {
            "xown": np.ascontiguousarray(xs), "vecown": np.ascontiguousarray(vs),
            "eidx": eidx[c], "evp": evp[c], "dstw": dstw[c], "rbtW": rbtW[c],
        }
        d.update(f)
        ins.append(d)
    return T, NT, ins


def kernel(**inputs):
    T, NT, ins = _make_inputs(inputs)
    nc = _build(NT, T)
    res = run_bass_kernel_spmd(nc, ins, list(range(8)))
    out = np.concatenate([res.results[c]["outT"][:3].T for c in range(8)], axis=0)
    return np.ascontiguousarray(out.astype(np.float32))
